# revision 35
# baseline (speedup 1.0000x reference)
"""Trainium2 Bass kernel for nn_ConcatLayer_37589553774933 (topk_masking).

Per-row computation on [N, 9] f32 (N = 8388608): three groups of 3
(up/none/down); per group a strict-argmax code in {-1,0,1}; a scalar
decision chain (calc/sign/idx); masking; probe-argmax group selection;
output [N, 3]. Rows are split evenly across 8 NeuronCores (SPMD).

Host path: the full [N, 9] array is handed directly to a jit-once
shard_map over 8 cores (each device gets a contiguous row block, no
host-side concat); the output buffer is a device-resident donated
array recycled from the previous call's output (the kernel writes
every output element, so stale contents are harmless), and the global
[N, 3] result is fetched with a single np.asarray.
"""


import os

os.environ.setdefault("JAX_PLATFORMS", "axon,cpu")

import numpy as np

import concourse.bass as bass
import concourse.mybir as mybir
from concourse.tile import TileContext

F32 = mybir.dt.float32
BF16 = mybir.dt.bfloat16
U8 = mybir.dt.uint8
OP = mybir.AluOpType

N_TOTAL = 8388608
N_CORES = 8
R_CORE = N_TOTAL // N_CORES
P = 128
ACT_SIGN_AB = os.environ.get("ACT_SIGN_AB", "1") == "1"
INTERLEAVE_FRONT = os.environ.get("INTERLEAVE_FRONT", "1") == "1"


def _copy_pred(eng, out, mask, data):
    return eng.add_instruction(
        mybir.InstCopyPredicated(
            name=f"I-{eng.bass.next_id()}",
            ins=[eng.lower_ap(mask, opt=False), eng.lower_ap(data, opt=False)],
            outs=[eng.lower_ap(out, opt=False)],
        )
    )


def split_multi_waits(nc, max_waits: int = 1):
    n_split = 0
    for f in nc.m.functions:
        for b in f.blocks:
            new_insts = []
            for ins in b.instructions:
                si = getattr(ins, "sync_info", None)
                if si and si.on_wait and len(si.on_wait) > max_waits:
                    waits = list(si.on_wait)
                    head, tail = waits[:-max_waits], waits[-max_waits:]
                    for k in range(0, len(head), max_waits):
                        chunk = head[k : k + max_waits]
                        nop = mybir.InstNoOp(
                            name=f"{ins.name}_waitsplit{k}",
                            ins=[],
                            outs=[],
                            sync_info=mybir.SyncInfo(
                                on_wait=list(chunk), on_update=[]
                            ),
                        )
                        nop.engine = ins.engine
                        new_insts.append(nop)
                    si.on_wait = tail
                    n_split += 1
                new_insts.append(ins)
            b.instructions = new_insts
    return n_split


def build_nc(R: int, C: int, bufs: int = 2):
    T = R // (P * C)
    assert R == T * P * C, (R, C)
    nc = bass.Bass("TRN2", debug=False)
    x = nc.declare_dram_parameter("x", [R, 9], F32, isOutput=False)
    o = nc.declare_dram_parameter("o", [R, 3], F32, isOutput=True)
    xt = x[:].rearrange("(t p c) j -> t p (c j)", t=T, p=P, c=C)
    ot = o[:].rearrange("(t p c) j -> t p (c j)", t=T, p=P, c=C)

    dve = nc.vector
    gp = nc.gpsimd
    act = nc.scalar

    with TileContext(nc) as tc:
        with tc.tile_pool(name="pool", bufs=bufs) as pool:
            for t in range(T):
                tin = pool.tile([P, 9 * C], F32, name="tin")
                nc.sync.dma_start(tin[:], xt[t])
                tv = tin[:].rearrange("p (c g j) -> p c g j", c=C, g=3, j=3)
                V = [tv[:, :, :, j] for j in range(3)]   # [P,C,3g] stride-3
                G = [tv[:, :, g, :] for g in range(3)]   # [P,C,3j] contig j

                # --- group codes ---------------------------------------
                X = pool.tile([P, 3 * C], F32, name="X")
                Xv = X[:].rearrange("p (c g) -> p c g", g=3)
                dve.tensor_tensor(Xv, V[1], V[2], op=OP.max)
                Y = pool.tile([P, 3 * C], F32, name="Y")
                Yv = Y[:].rearrange("p (c g) -> p c g", g=3)
                dve.tensor_tensor(Yv, V[0], V[1], op=OP.max)

                DA = pool.tile([P, 3 * C], F32, name="DA")
                DAv = DA[:].rearrange("p (c g) -> p c g", g=3)
                gp.tensor_tensor(DAv, V[0], Xv, op=OP.subtract)
                DB = pool.tile([P, 3 * C], F32, name="DB")
                DBv = DB[:].rearrange("p (c g) -> p c g", g=3)
                gp.tensor_tensor(DBv, V[2], Yv, op=OP.subtract)

                A = pool.tile([P, 3 * C], BF16, name="A")
                dve.tensor_scalar(A[:], DA[:], 0.0, None, op0=OP.is_gt)
                Bt = pool.tile([P, 3 * C], BF16, name="Bt")
                dve.tensor_scalar(Bt[:], DB[:], 0.0, None, op0=OP.is_gt)
                M = pool.tile([P, 3 * C], BF16, name="M")
                dve.tensor_tensor(M[:], A[:], Bt[:], op=OP.subtract)
                Mv = M[:].rearrange("p (c g) -> p c g", g=3)
                mu, mn, md = Mv[:, :, 0], Mv[:, :, 1], Mv[:, :, 2]

                # --- calc = |mn| * (mu + md + mn) ----------------------
                S1 = pool.tile([P, C], BF16, name="S1")
                dve.tensor_tensor(S1[:], mu, md, op=OP.add)
                S2 = pool.tile([P, C], BF16, name="S2")
                dve.tensor_tensor(S2[:], S1[:], mn, op=OP.add)
                T1 = pool.tile([P, C], BF16, name="T1")
                dve.tensor_tensor(T1[:], mn, S2[:], op=OP.mult)
                CALC = pool.tile([P, C], BF16, name="CALC")
                dve.tensor_tensor(CALC[:], mn, T1[:], op=OP.mult)

                SGN = pool.tile([P, C], BF16, name="SGN")
                act.sign(SGN[:], CALC[:])
                E0 = pool.tile([P, C], U8, name="E0")
                dve.tensor_scalar(E0[:], CALC[:], 1.0, None, op0=OP.is_equal)
                E1 = pool.tile([P, C], U8, name="E1")
                dve.tensor_scalar(E1[:], CALC[:], 0.0, None, op0=OP.is_equal)

                # --- keep_g = (m_g == sgn) -----------------------------
                KD = pool.tile([P, 3 * C], BF16, name="KD")
                KDv = KD[:].rearrange("p (c g) -> p c g", g=3)
                sgnb = SGN[:].broadcast_to((P, C, 3))
                gp.tensor_tensor(KDv, Mv, sgnb, op=OP.subtract)
                KEEP = pool.tile([P, 3 * C], F32, name="KEEP")
                KEEPv = KEEP[:].rearrange("p (c g) -> p c g", g=3)
                dve.tensor_scalar(KEEP[:], KD[:], 0.0, None, op0=OP.is_equal)

                # --- probe ---------------------------------------------
                PRraw = pool.tile([P, 3 * C], F32, name="PRraw")
                PRrawv = PRraw[:].rearrange("p (c g) -> p c g", g=3)
                act.copy(PRrawv, V[2])
                e1b = E1[:].broadcast_to((P, C, 3))
                e0b = E0[:].broadcast_to((P, C, 3))
                _copy_pred(dve, PRrawv, e1b, V[1])
                _copy_pred(dve, PRrawv, e0b, V[0])
                PR = pool.tile([P, 3 * C], F32, name="PR")
                PRv = PR[:].rearrange("p (c g) -> p c g", g=3)
                gp.tensor_tensor(PR[:], PRraw[:], KEEP[:], op=OP.mult)

                # --- choice --------------------------------------------
                CN = pool.tile([P, C], U8, name="CN")
                dve.tensor_tensor(CN[:], PRv[:, :, 1], PRv[:, :, 2], op=OP.is_ge)
                MND = pool.tile([P, C], F32, name="MND")
                dve.tensor_tensor(MND[:], PRv[:, :, 1], PRv[:, :, 2], op=OP.max)
                CU = pool.tile([P, C], U8, name="CU")
                dve.tensor_tensor(CU[:], PRv[:, :, 0], MND[:], op=OP.is_ge)
                cnb = CN[:].broadcast_to((P, C, 3))
                cub = CU[:].broadcast_to((P, C, 3))

                # --- output --------------------------------------------
                OTraw = pool.tile([P, 3 * C], F32, name="OTraw")
                OTrawv = OTraw[:].rearrange("p (c j) -> p c j", j=3)
                act.copy(OTrawv, G[2])
                _copy_pred(dve, OTrawv, cnb, G[1])
                _copy_pred(dve, OTrawv, cub, G[0])

                KSEL = pool.tile([P, C], F32, name="KSEL")
                act.copy(KSEL[:], KEEPv[:, :, 2])
                _copy_pred(dve, KSEL[:], CN[:], KEEPv[:, :, 1])
                _copy_pred(dve, KSEL[:], CU[:], KEEPv[:, :, 0])

                OT = pool.tile([P, 3 * C], F32, name="OT")
                OTv = OT[:].rearrange("p (c j) -> p c j", j=3)
                kselb = KSEL[:].broadcast_to((P, C, 3))
                gp.tensor_tensor(OTv, OTrawv, kselb, op=OP.mult)

                nc.sync.dma_start(ot[t], OT[:])

    return nc


def build_nc_v2(R: int, C: int, bufs: int = 3):
    """Planar-bf16 variant: A/B/M/KD/E masks stored planar (g-major) and
    packed so bf16 DVE ops hit the 2x/4x perf modes; KEEP planar f32 so
    its per-group planes are contiguous for the KSEL selects. Engine
    split mirrors v1 (Pool: subs+mask-mults, ACT: base copies + sign,
    DVE: maxes, compares, predicated selects)."""
    T = R // (P * C)
    assert R == T * P * C, (R, C)
    nc = bass.Bass("TRN2", debug=False)
    x = nc.declare_dram_parameter("x", [R, 9], F32, isOutput=False)
    o = nc.declare_dram_parameter("o", [R, 3], F32, isOutput=True)
    xt = x[:].rearrange("(t p c) j -> t p (c j)", t=T, p=P, c=C)
    ot = o[:].rearrange("(t p c) j -> t p (c j)", t=T, p=P, c=C)

    dve = nc.vector
    gp = nc.gpsimd
    act = nc.scalar

    with TileContext(nc) as tc:
        with tc.tile_pool(name="pool", bufs=bufs) as pool:
            for t in range(T):
                tin = pool.tile([P, 9 * C], F32, name="tin")
                nc.sync.dma_start(tin[:], xt[t])
                tv = tin[:].rearrange("p (c g j) -> p c g j", c=C, g=3, j=3)
                V = [tv[:, :, :, j] for j in range(3)]   # [P,C,3g] stride-3
                G = [tv[:, :, g, :] for g in range(3)]   # [P,C,3j] contig j

                # --- group codes (planar bf16 bits) --------------------
                X = pool.tile([P, 3 * C], F32, name="X")
                Xv = X[:].rearrange("p (c g) -> p c g", g=3)
                dve.tensor_tensor(Xv, V[1], V[2], op=OP.max)
                Y = pool.tile([P, 3 * C], F32, name="Y")
                Yv = Y[:].rearrange("p (c g) -> p c g", g=3)
                dve.tensor_tensor(Yv, V[0], V[1], op=OP.max)

                DA = pool.tile([P, 3 * C], F32, name="DA")
                DAv = DA[:].rearrange("p (c g) -> p c g", g=3)
                gp.tensor_tensor(DAv, V[0], Xv, op=OP.subtract)
                DB = pool.tile([P, 3 * C], F32, name="DB")
                DBv = DB[:].rearrange("p (c g) -> p c g", g=3)
                gp.tensor_tensor(DBv, V[2], Yv, op=OP.subtract)

                # planar (g-major) bf16 bit planes, packed along c
                A = pool.tile([P, 3 * C], BF16, name="A")
                A_gc = A[:].rearrange("p (g c) -> p g c", g=3)
                DA_gc = DA[:].rearrange("p (c g) -> p g c", g=3)
                B = pool.tile([P, 3 * C], BF16, name="B")
                B_gc = B[:].rearrange("p (g c) -> p g c", g=3)
                DB_gc = DB[:].rearrange("p (c g) -> p g c", g=3)
                if ACT_SIGN_AB:
                    # sign on ACT, then relu via cheap packed-bf16 ts-max on
                    # DVE: relu(sign(d)) == (d > 0) exactly.
                    SA = pool.tile([P, 3 * C], BF16, name="SA")
                    SA_gc = SA[:].rearrange("p (g c) -> p g c", g=3)
                    act.sign(SA_gc, DA_gc)
                    dve.tensor_scalar(A[:], SA[:], 0.0, None, op0=OP.max)
                    SB = pool.tile([P, 3 * C], BF16, name="SB")
                    SB_gc = SB[:].rearrange("p (g c) -> p g c", g=3)
                    act.sign(SB_gc, DB_gc)
                    dve.tensor_scalar(B[:], SB[:], 0.0, None, op0=OP.max)
                else:
                    dve.tensor_scalar(A_gc, DA_gc, 0.0, None, op0=OP.is_gt)
                    dve.tensor_scalar(B_gc, DB_gc, 0.0, None, op0=OP.is_gt)

                M = pool.tile([P, 3 * C], BF16, name="M")
                dve.tensor_tensor(M[:], A[:], B[:], op=OP.subtract)
                mu, mn, md = (M[:, 0:C], M[:, C : 2 * C], M[:, 2 * C : 3 * C])

                # --- calc = |mn| * (mu + md + mn) (packed bf16) --------
                S1 = pool.tile([P, C], BF16, name="S1")
                dve.tensor_tensor(S1[:], mu, md, op=OP.add)
                S2 = pool.tile([P, C], BF16, name="S2")
                dve.tensor_tensor(S2[:], S1[:], mn, op=OP.add)
                T1 = pool.tile([P, C], BF16, name="T1")
                dve.tensor_tensor(T1[:], mn, S2[:], op=OP.mult)
                CALC = pool.tile([P, C], BF16, name="CALC")
                dve.tensor_tensor(CALC[:], mn, T1[:], op=OP.mult)

                SGN = pool.tile([P, C], BF16, name="SGN")
                act.sign(SGN[:], CALC[:])
                E0 = pool.tile([P, C], U8, name="E0")
                dve.tensor_scalar(E0[:], CALC[:], 1.0, None, op0=OP.is_equal)
                E1 = pool.tile([P, C], U8, name="E1")
                dve.tensor_scalar(E1[:], CALC[:], 0.0, None, op0=OP.is_equal)

                # --- keep_g = (m_g == sgn), planar f32 planes ----------
                KD = pool.tile([P, 3 * C], BF16, name="KD")
                KD_cg = KD[:].rearrange("p (g c) -> p c g", g=3)
                sgn_b = SGN[:].broadcast_to((P, C, 3))
                M_cg = M[:].rearrange("p (g c) -> p c g", g=3)
                gp.tensor_tensor(KD_cg, M_cg, sgn_b, op=OP.subtract)
                KEEP = pool.tile([P, 3 * C], F32, name="KEEP")
                dve.tensor_scalar(KEEP[:], KD[:], 0.0, None, op0=OP.is_equal)
                keep_u, keep_n, keep_d = (
                    KEEP[:, 0:C],
                    KEEP[:, C : 2 * C],
                    KEEP[:, 2 * C : 3 * C],
                )
                KEEP_cg = KEEP[:].rearrange("p (g c) -> p c g", g=3)

                # --- probe ---------------------------------------------
                PRraw = pool.tile([P, 3 * C], F32, name="PRraw")
                PRrawv = PRraw[:].rearrange("p (c g) -> p c g", g=3)
                act.copy(PRrawv, V[2])
                e1b = E1[:].broadcast_to((P, C, 3))
                e0b = E0[:].broadcast_to((P, C, 3))
                dve.copy_predicated(PRrawv, e1b, V[1])
                dve.copy_predicated(PRrawv, e0b, V[0])
                PR = pool.tile([P, 3 * C], F32, name="PR")
                PRv = PR[:].rearrange("p (c g) -> p c g", g=3)
                gp.tensor_tensor(PRv, PRrawv, KEEP_cg, op=OP.mult)

                # --- choice --------------------------------------------
                CN = pool.tile([P, C], U8, name="CN")
                dve.tensor_tensor(CN[:], PRv[:, :, 1], PRv[:, :, 2], op=OP.is_ge)
                MND = pool.tile([P, C], F32, name="MND")
                dve.tensor_tensor(MND[:], PRv[:, :, 1], PRv[:, :, 2], op=OP.max)
                CU = pool.tile([P, C], U8, name="CU")
                dve.tensor_tensor(CU[:], PRv[:, :, 0], MND[:], op=OP.is_ge)
                cnb = CN[:].broadcast_to((P, C, 3))
                cub = CU[:].broadcast_to((P, C, 3))

                # --- output --------------------------------------------
                OTraw = pool.tile([P, 3 * C], F32, name="OTraw")
                OTrawv = OTraw[:].rearrange("p (c j) -> p c j", j=3)
                act.copy(OTrawv, G[2])
                dve.copy_predicated(OTrawv, cnb, G[1])
                dve.copy_predicated(OTrawv, cub, G[0])

                KSEL = pool.tile([P, C], F32, name="KSEL")
                act.copy(KSEL[:], keep_d)
                dve.copy_predicated(KSEL[:], CN[:], keep_n)
                dve.copy_predicated(KSEL[:], CU[:], keep_u)

                OT = pool.tile([P, 3 * C], F32, name="OT")
                OTv = OT[:].rearrange("p (c j) -> p c j", j=3)
                kselb = KSEL[:].broadcast_to((P, C, 3))
                gp.tensor_tensor(OTv, OTrawv, kselb, op=OP.mult)

                nc.sync.dma_start(ot[t], OT[:])

    return nc


def build_nc_v3(R: int, C: int, bufs: int = 4):
    """v2 with software-pipelined emission: stage S0 (DMA + codes) of
    tiles t+1/t+2 is emitted before stages S1/S2 of tile t, so in-order
    engines have upstream work queued while a tile waits on cross-engine
    hops (Pool PR-mult -> DVE compares etc.). Buffer ring (bufs) must
    cover the 3-stage lifetime."""
    T = R // (P * C)
    assert R == T * P * C, (R, C)
    nc = bass.Bass("TRN2", debug=False)
    x = nc.declare_dram_parameter("x", [R, 9], F32, isOutput=False)
    o = nc.declare_dram_parameter("o", [R, 3], F32, isOutput=True)
    xt = x[:].rearrange("(t p c) j -> t p (c j)", t=T, p=P, c=C)
    ot = o[:].rearrange("(t p c) j -> t p (c j)", t=T, p=P, c=C)

    dve = nc.vector
    gp = nc.gpsimd
    act = nc.scalar

    with TileContext(nc) as tc:
        with tc.tile_pool(name="pool", bufs=bufs) as pool:

            def emit_s0(t):
                s = {}
                tin = pool.tile([P, 9 * C], F32, name="tin")
                nc.sync.dma_start(tin[:], xt[t])
                tv = tin[:].rearrange("p (c g j) -> p c g j", c=C, g=3, j=3)
                s["V"] = [tv[:, :, :, j] for j in range(3)]
                s["G"] = [tv[:, :, g, :] for g in range(3)]
                V = s["V"]

                X = pool.tile([P, 3 * C], F32, name="X")
                Xv = X[:].rearrange("p (c g) -> p c g", g=3)
                dve.tensor_tensor(Xv, V[1], V[2], op=OP.max)
                Y = pool.tile([P, 3 * C], F32, name="Y")
                Yv = Y[:].rearrange("p (c g) -> p c g", g=3)
                dve.tensor_tensor(Yv, V[0], V[1], op=OP.max)

                DA = pool.tile([P, 3 * C], F32, name="DA")
                DAv = DA[:].rearrange("p (c g) -> p c g", g=3)
                gp.tensor_tensor(DAv, V[0], Xv, op=OP.subtract)
                DB = pool.tile([P, 3 * C], F32, name="DB")
                DBv = DB[:].rearrange("p (c g) -> p c g", g=3)
                gp.tensor_tensor(DBv, V[2], Yv, op=OP.subtract)

                A = pool.tile([P, 3 * C], BF16, name="A")
                DA_gc = DA[:].rearrange("p (c g) -> p g c", g=3)
                B = pool.tile([P, 3 * C], BF16, name="B")
                DB_gc = DB[:].rearrange("p (c g) -> p g c", g=3)
                SA = pool.tile([P, 3 * C], BF16, name="SA")
                SA_gc = SA[:].rearrange("p (g c) -> p g c", g=3)
                act.sign(SA_gc, DA_gc)
                dve.tensor_scalar(A[:], SA[:], 0.0, None, op0=OP.max)
                SB = pool.tile([P, 3 * C], BF16, name="SB")
                SB_gc = SB[:].rearrange("p (g c) -> p g c", g=3)
                act.sign(SB_gc, DB_gc)
                dve.tensor_scalar(B[:], SB[:], 0.0, None, op0=OP.max)

                M = pool.tile([P, 3 * C], BF16, name="M")
                dve.tensor_tensor(M[:], A[:], B[:], op=OP.subtract)
                s["M"] = M
                s["t"] = t
                return s

            def emit_s1(s):
                M = s["M"]
                V = s["V"]
                mu, mn, md = (
                    M[:, 0:C],
                    M[:, C : 2 * C],
                    M[:, 2 * C : 3 * C],
                )
                S1 = pool.tile([P, C], BF16, name="S1")
                dve.tensor_tensor(S1[:], mu, md, op=OP.add)
                S2 = pool.tile([P, C], BF16, name="S2")
                dve.tensor_tensor(S2[:], S1[:], mn, op=OP.add)
                T1 = pool.tile([P, C], BF16, name="T1")
                dve.tensor_tensor(T1[:], mn, S2[:], op=OP.mult)
                CALC = pool.tile([P, C], BF16, name="CALC")
                dve.tensor_tensor(CALC[:], mn, T1[:], op=OP.mult)

                SGN = pool.tile([P, C], BF16, name="SGN")
                act.sign(SGN[:], CALC[:])
                E0 = pool.tile([P, C], U8, name="E0")
                dve.tensor_scalar(E0[:], CALC[:], 1.0, None, op0=OP.is_equal)
                E1 = pool.tile([P, C], U8, name="E1")
                dve.tensor_scalar(E1[:], CALC[:], 0.0, None, op0=OP.is_equal)

                KD = pool.tile([P, 3 * C], BF16, name="KD")
                KD_cg = KD[:].rearrange("p (g c) -> p c g", g=3)
                sgn_b = SGN[:].broadcast_to((P, C, 3))
                M_cg = M[:].rearrange("p (g c) -> p c g", g=3)
                gp.tensor_tensor(KD_cg, M_cg, sgn_b, op=OP.subtract)
                KEEP = pool.tile([P, 3 * C], F32, name="KEEP")
                dve.tensor_scalar(KEEP[:], KD[:], 0.0, None, op0=OP.is_equal)
                s["KEEP"] = KEEP

                PRraw = pool.tile([P, 3 * C], F32, name="PRraw")
                PRrawv = PRraw[:].rearrange("p (c g) -> p c g", g=3)
                act.copy(PRrawv, V[2])
                e1b = E1[:].broadcast_to((P, C, 3))
                e0b = E0[:].broadcast_to((P, C, 3))
                dve.copy_predicated(PRrawv, e1b, V[1])
                dve.copy_predicated(PRrawv, e0b, V[0])
                PR = pool.tile([P, 3 * C], F32, name="PR")
                KEEP_cg = KEEP[:].rearrange("p (g c) -> p c g", g=3)
                PRv = PR[:].rearrange("p (c g) -> p c g", g=3)
                gp.tensor_tensor(PRv, PRrawv, KEEP_cg, op=OP.mult)
                s["PR"] = PR

            def emit_s2(s):
                PR = s["PR"]
                KEEP = s["KEEP"]
                G = s["G"]
                t = s["t"]
                PRv = PR[:].rearrange("p (c g) -> p c g", g=3)
                keep_u, keep_n, keep_d = (
                    KEEP[:, 0:C],
                    KEEP[:, C : 2 * C],
                    KEEP[:, 2 * C : 3 * C],
                )
                CN = pool.tile([P, C], U8, name="CN")
                dve.tensor_tensor(
                    CN[:], PRv[:, :, 1], PRv[:, :, 2], op=OP.is_ge
                )
                MND = pool.tile([P, C], F32, name="MND")
                dve.tensor_tensor(
                    MND[:], PRv[:, :, 1], PRv[:, :, 2], op=OP.max
                )
                CU = pool.tile([P, C], U8, name="CU")
                dve.tensor_tensor(CU[:], PRv[:, :, 0], MND[:], op=OP.is_ge)
                cnb = CN[:].broadcast_to((P, C, 3))
                cub = CU[:].broadcast_to((P, C, 3))

                OTraw = pool.tile([P, 3 * C], F32, name="OTraw")
                OTrawv = OTraw[:].rearrange("p (c j) -> p c j", j=3)
                act.copy(OTrawv, G[2])
                dve.copy_predicated(OTrawv, cnb, G[1])
                dve.copy_predicated(OTrawv, cub, G[0])

                KSEL = pool.tile([P, C], F32, name="KSEL")
                act.copy(KSEL[:], keep_d)
                dve.copy_predicated(KSEL[:], CN[:], keep_n)
                dve.copy_predicated(KSEL[:], CU[:], keep_u)

                OT = pool.tile([P, 3 * C], F32, name="OT")
                OTv = OT[:].rearrange("p (c j) -> p c j", j=3)
                kselb = KSEL[:].broadcast_to((P, C, 3))
                gp.tensor_tensor(OTv, OTrawv, kselb, op=OP.mult)
                nc.sync.dma_start(ot[t], OT[:])

            state = {}
            for t in range(T):
                state[t] = emit_s0(t)
                if t - 1 in state:
                    emit_s1(state[t - 1])
                if t - 2 in state:
                    emit_s2(state.pop(t - 2))
            if T - 1 in state:
                emit_s1(state[T - 1])
            if T - 2 in state:
                emit_s2(state.pop(T - 2))
            if T - 1 in state:
                emit_s2(state.pop(T - 1))

    return nc


def build_nc_v4(R: int, C: int, bufs: int = 4):
    """v2 with the codes front (DMA, X/Y maxes, Pool subs, ACT signs)
    software-pipelined one tile ahead and the input DMA prefetched two
    tiles ahead: when the in-order DVE stream reaches tile t's A/B bits,
    the cross-engine chain X/Y(t) -> Pool DA/DB(t) -> ACT SA/SB(t) has
    had a full iteration to drain, removing the per-tile DVE bubble."""
    T = R // (P * C)
    assert R == T * P * C, (R, C)
    nc = bass.Bass("TRN2", debug=False)
    x = nc.declare_dram_parameter("x", [R, 9], F32, isOutput=False)
    o = nc.declare_dram_parameter("o", [R, 3], F32, isOutput=True)
    xt = x[:].rearrange("(t p c) j -> t p (c j)", t=T, p=P, c=C)
    ot = o[:].rearrange("(t p c) j -> t p (c j)", t=T, p=P, c=C)

    dve = nc.vector
    gp = nc.gpsimd
    act = nc.scalar

    with TileContext(nc) as tc:
        with tc.tile_pool(name="pool", bufs=bufs) as pool:

            def emit_dma(t):
                tin = pool.tile([P, 9 * C], F32, name="tin")
                nc.sync.dma_start(tin[:], xt[t])
                return tin

            def make_views(t, tin):
                s = {"t": t}
                tv = tin[:].rearrange("p (c g j) -> p c g j", c=C, g=3, j=3)
                s["V"] = [tv[:, :, :, j] for j in range(3)]
                s["G"] = [tv[:, :, g, :] for g in range(3)]
                return s

            def emit_x(s):
                V = s["V"]
                X = pool.tile([P, 3 * C], F32, name="X")
                Xv = X[:].rearrange("p (c g) -> p c g", g=3)
                dve.tensor_tensor(Xv, V[1], V[2], op=OP.max)
                s["Xv"] = Xv

            def emit_y(s):
                V = s["V"]
                Y = pool.tile([P, 3 * C], F32, name="Y")
                Yv = Y[:].rearrange("p (c g) -> p c g", g=3)
                dve.tensor_tensor(Yv, V[0], V[1], op=OP.max)
                s["Yv"] = Yv

            def emit_subs_signs(s):
                V = s["V"]
                DA = pool.tile([P, 3 * C], F32, name="DA")
                DAv = DA[:].rearrange("p (c g) -> p c g", g=3)
                gp.tensor_tensor(DAv, V[0], s["Xv"], op=OP.subtract)
                DB = pool.tile([P, 3 * C], F32, name="DB")
                DBv = DB[:].rearrange("p (c g) -> p c g", g=3)
                gp.tensor_tensor(DBv, V[2], s["Yv"], op=OP.subtract)

                SA = pool.tile([P, 3 * C], BF16, name="SA")
                SA_gc = SA[:].rearrange("p (g c) -> p g c", g=3)
                act.sign(SA_gc, DA[:].rearrange("p (c g) -> p g c", g=3))
                SB = pool.tile([P, 3 * C], BF16, name="SB")
                SB_gc = SB[:].rearrange("p (g c) -> p g c", g=3)
                act.sign(SB_gc, DB[:].rearrange("p (c g) -> p g c", g=3))
                s["SA"], s["SB"] = SA, SB

            def emit_front(t, tin):
                s = make_views(t, tin)
                emit_x(s)
                emit_y(s)
                emit_subs_signs(s)
                return s

            def emit_rest_a(s, nxt=None):
                V = s["V"]
                A = pool.tile([P, 3 * C], BF16, name="A")
                dve.tensor_scalar(A[:], s["SA"][:], 0.0, None, op0=OP.max)
                B = pool.tile([P, 3 * C], BF16, name="B")
                dve.tensor_scalar(B[:], s["SB"][:], 0.0, None, op0=OP.max)
                M = pool.tile([P, 3 * C], BF16, name="M")
                dve.tensor_tensor(M[:], A[:], B[:], op=OP.subtract)
                mu, mn, md = (
                    M[:, 0:C],
                    M[:, C : 2 * C],
                    M[:, 2 * C : 3 * C],
                )

                S1 = pool.tile([P, C], BF16, name="S1")
                dve.tensor_tensor(S1[:], mu, md, op=OP.add)
                if nxt is not None:
                    # weave the next tile's X max between serial chain
                    # ops: independent DVE work hides the sem-ack gaps
                    # and starts the next Pool->ACT sign chain earlier.
                    emit_x(nxt)
                S2 = pool.tile([P, C], BF16, name="S2")
                dve.tensor_tensor(S2[:], S1[:], mn, op=OP.add)
                T1 = pool.tile([P, C], BF16, name="T1")
                dve.tensor_tensor(T1[:], mn, S2[:], op=OP.mult)
                if nxt is not None:
                    emit_y(nxt)
                CALC = pool.tile([P, C], BF16, name="CALC")
                dve.tensor_tensor(CALC[:], mn, T1[:], op=OP.mult)

                SGN = pool.tile([P, C], BF16, name="SGN")
                act.sign(SGN[:], CALC[:])

                E0 = pool.tile([P, C], U8, name="E0")
                dve.tensor_scalar(E0[:], CALC[:], 1.0, None, op0=OP.is_equal)
                E1 = pool.tile([P, C], U8, name="E1")
                dve.tensor_scalar(E1[:], CALC[:], 0.0, None, op0=OP.is_equal)

                PRraw = pool.tile([P, 3 * C], F32, name="PRraw")
                PRrawv = PRraw[:].rearrange("p (c g) -> p c g", g=3)
                act.copy(PRrawv, V[2])
                e1b = E1[:].broadcast_to((P, C, 3))
                e0b = E0[:].broadcast_to((P, C, 3))
                dve.copy_predicated(PRrawv, e1b, V[1])
                dve.copy_predicated(PRrawv, e0b, V[0])

                # keep = (m == sgn) in one DVE tt — no Pool round-trip
                # on the critical path to the PR mask.
                KEEP = pool.tile([P, 3 * C], F32, name="KEEP")
                KEEP_cg2 = KEEP[:].rearrange("p (g c) -> p c g", g=3)
                sgn_b = SGN[:].broadcast_to((P, C, 3))
                M_cg = M[:].rearrange("p (g c) -> p c g", g=3)
                dve.tensor_tensor(KEEP_cg2, M_cg, sgn_b, op=OP.is_equal)

                PR = pool.tile([P, 3 * C], F32, name="PR")
                PRv = PR[:].rearrange("p (c g) -> p c g", g=3)
                KEEP_cg = KEEP[:].rearrange("p (g c) -> p c g", g=3)
                gp.tensor_tensor(PRv, PRrawv, KEEP_cg, op=OP.mult)
                s["KEEP"], s["PRv"] = KEEP, PRv

            def emit_rest_b(s):
                t = s["t"]
                G = s["G"]
                KEEP = s["KEEP"]
                PRv = s["PRv"]
                keep_u, keep_n, keep_d = (
                    KEEP[:, 0:C],
                    KEEP[:, C : 2 * C],
                    KEEP[:, 2 * C : 3 * C],
                )
                CN = pool.tile([P, C], U8, name="CN")
                dve.tensor_tensor(
                    CN[:], PRv[:, :, 1], PRv[:, :, 2], op=OP.is_ge
                )
                MND = pool.tile([P, C], F32, name="MND")
                dve.tensor_tensor(
                    MND[:], PRv[:, :, 1], PRv[:, :, 2], op=OP.max
                )
                CU = pool.tile([P, C], U8, name="CU")
                dve.tensor_tensor(CU[:], PRv[:, :, 0], MND[:], op=OP.is_ge)
                cnb = CN[:].broadcast_to((P, C, 3))
                cub = CU[:].broadcast_to((P, C, 3))

                OTraw = pool.tile([P, 3 * C], F32, name="OTraw")
                OTrawv = OTraw[:].rearrange("p (c j) -> p c j", j=3)
                act.copy(OTrawv, G[2])
                dve.copy_predicated(OTrawv, cnb, G[1])
                dve.copy_predicated(OTrawv, cub, G[0])

                KSEL = pool.tile([P, C], F32, name="KSEL")
                act.copy(KSEL[:], keep_d)
                dve.copy_predicated(KSEL[:], CN[:], keep_n)
                dve.copy_predicated(KSEL[:], CU[:], keep_u)

                OT = pool.tile([P, 3 * C], F32, name="OT")
                OTv = OT[:].rearrange("p (c j) -> p c j", j=3)
                kselb = KSEL[:].broadcast_to((P, C, 3))
                gp.tensor_tensor(OTv, OTrawv, kselb, op=OP.mult)
                nc.sync.dma_start(ot[t], OT[:])

            tins = {}
            fronts = {}
            if INTERLEAVE_FRONT:
                # v5 ordering: rest_a(t) fills the Pool KD latency with
                # the PRraw cpreds; front(t+1)'s X/Y maxes fill the Pool
                # PR-mult latency before rest_b(t)'s CN/CU compares.
                tins[0] = emit_dma(0)
                fronts[0] = emit_front(0, tins.pop(0))
                if T > 1:
                    tins[1] = emit_dma(1)
                for t in range(T):
                    if t + 2 < T:
                        tins[t + 2] = emit_dma(t + 2)
                    s = fronts.pop(t)
                    if t + 1 in tins:
                        nxt = make_views(t + 1, tins.pop(t + 1))
                        emit_rest_a(s, nxt=nxt)
                        emit_subs_signs(nxt)
                        fronts[t + 1] = nxt
                    else:
                        emit_rest_a(s)
                    emit_rest_b(s)
            else:
                for t in range(T):
                    if t == 0:
                        tins[0] = emit_dma(0)
                        fronts[0] = emit_front(0, tins.pop(0))
                        if T > 1:
                            tins[1] = emit_dma(1)
                    if t + 2 < T:
                        tins[t + 2] = emit_dma(t + 2)
                    if t + 1 in tins:
                        fronts[t + 1] = emit_front(t + 1, tins.pop(t + 1))
                    s = fronts.pop(t)
                    emit_rest_a(s)
                    emit_rest_b(s)

    return nc


def build_nc_v5(R: int, C: int, bufs: int = 4):
    """v4 with the X/Y maxes eliminated: per-pair signs s01=sign(v0-v1),
    s02=sign(v0-v2), s21=sign(v2-v1) are produced by Pool subs + ACT
    signs (one tile ahead), and the group code becomes pure packed-bf16
    DVE arithmetic: m = relu(s01+s02-1) - relu(s21-s02-1), each relu a
    single fused two-op tensor_scalar. Stage order per iteration t:
    dma(t+2) | front(t+1): subs+signs | rest_a(t): selects+KEEP+PR |
    mid(t+1): codes+chain (fills the Pool PR(t) latency on DVE) |
    rest_b(t): choice+output."""
    T = R // (P * C)
    assert R == T * P * C, (R, C)
    nc = bass.Bass("TRN2", debug=False)
    x = nc.declare_dram_parameter("x", [R, 9], F32, isOutput=False)
    o = nc.declare_dram_parameter("o", [R, 3], F32, isOutput=True)
    xt = x[:].rearrange("(t p c) j -> t p (c j)", t=T, p=P, c=C)
    ot = o[:].rearrange("(t p c) j -> t p (c j)", t=T, p=P, c=C)

    dve = nc.vector
    gp = nc.gpsimd
    act = nc.scalar

    with TileContext(nc) as tc:
        with tc.tile_pool(name="pool", bufs=bufs) as pool:

            def emit_dma(t):
                tin = pool.tile([P, 9 * C], F32, name="tin")
                nc.sync.dma_start(tin[:], xt[t])
                return tin

            def emit_front(t, tin):
                s = {"t": t}
                tv = tin[:].rearrange("p (c g j) -> p c g j", c=C, g=3, j=3)
                s["V"] = [tv[:, :, :, j] for j in range(3)]
                s["G"] = [tv[:, :, g, :] for g in range(3)]
                V = s["V"]

                sigs = []
                for nm, (a, b) in (
                    ("01", (V[0], V[1])),
                    ("02", (V[0], V[2])),
                    ("21", (V[2], V[1])),
                ):
                    D = pool.tile([P, 3 * C], F32, name=f"D{nm}")
                    Dv = D[:].rearrange("p (c g) -> p c g", g=3)
                    gp.tensor_tensor(Dv, a, b, op=OP.subtract)
                    S = pool.tile([P, 3 * C], BF16, name=f"S{nm}")
                    S_gc = S[:].rearrange("p (g c) -> p g c", g=3)
                    act.sign(S_gc, D[:].rearrange("p (c g) -> p g c", g=3))
                    sigs.append(S)
                s["s01"], s["s02"], s["s21"] = sigs
                return s

            def emit_mid(s):
                # group codes + calc chain, all packed bf16 on DVE
                s01, s02, s21 = s["s01"], s["s02"], s["s21"]
                U = pool.tile([P, 3 * C], BF16, name="U")
                dve.tensor_tensor(U[:], s01[:], s02[:], op=OP.add)
                W = pool.tile([P, 3 * C], BF16, name="W")
                dve.tensor_tensor(W[:], s21[:], s02[:], op=OP.subtract)
                RU = pool.tile([P, 3 * C], BF16, name="RU")
                dve.tensor_scalar(
                    RU[:], U[:], -1.0, 0.0, op0=OP.add, op1=OP.max
                )
                M = pool.tile([P, 3 * C], BF16, name="M")
                # m = relu(u-1) - relu(w-1): fold the second relu via
                # rw = max(w-1, 0) then m = ru - rw
                RW = pool.tile([P, 3 * C], BF16, name="RW")
                dve.tensor_scalar(
                    RW[:], W[:], -1.0, 0.0, op0=OP.add, op1=OP.max
                )
                dve.tensor_tensor(M[:], RU[:], RW[:], op=OP.subtract)
                mu, mn, md = (
                    M[:, 0:C],
                    M[:, C : 2 * C],
                    M[:, 2 * C : 3 * C],
                )

                S1 = pool.tile([P, C], BF16, name="S1")
                dve.tensor_tensor(S1[:], mu, md, op=OP.add)
                S2 = pool.tile([P, C], BF16, name="S2")
                dve.tensor_tensor(S2[:], S1[:], mn, op=OP.add)
                T1 = pool.tile([P, C], BF16, name="T1")
                dve.tensor_tensor(T1[:], mn, S2[:], op=OP.mult)
                CALC = pool.tile([P, C], BF16, name="CALC")
                dve.tensor_tensor(CALC[:], mn, T1[:], op=OP.mult)

                SGN = pool.tile([P, C], BF16, name="SGN")
                act.sign(SGN[:], CALC[:])
                E0 = pool.tile([P, C], U8, name="E0")
                dve.tensor_scalar(E0[:], CALC[:], 1.0, None, op0=OP.is_equal)
                E1 = pool.tile([P, C], U8, name="E1")
                dve.tensor_scalar(E1[:], CALC[:], 0.0, None, op0=OP.is_equal)
                s["M"], s["SGN"], s["E0"], s["E1"] = M, SGN, E0, E1

            def emit_rest_a(s):
                V = s["V"]
                M, SGN, E0, E1 = s["M"], s["SGN"], s["E0"], s["E1"]

                PRraw = pool.tile([P, 3 * C], F32, name="PRraw")
                PRrawv = PRraw[:].rearrange("p (c g) -> p c g", g=3)
                act.copy(PRrawv, V[2])
                e1b = E1[:].broadcast_to((P, C, 3))
                e0b = E0[:].broadcast_to((P, C, 3))
                dve.copy_predicated(PRrawv, e1b, V[1])
                dve.copy_predicated(PRrawv, e0b, V[0])

                KEEP = pool.tile([P, 3 * C], F32, name="KEEP")
                KEEP_cg2 = KEEP[:].rearrange("p (g c) -> p c g", g=3)
                sgn_b = SGN[:].broadcast_to((P, C, 3))
                M_cg = M[:].rearrange("p (g c) -> p c g", g=3)
                dve.tensor_tensor(KEEP_cg2, M_cg, sgn_b, op=OP.is_equal)

                PR = pool.tile([P, 3 * C], F32, name="PR")
                PRv = PR[:].rearrange("p (c g) -> p c g", g=3)
                KEEP_cg = KEEP[:].rearrange("p (g c) -> p c g", g=3)
                gp.tensor_tensor(PRv, PRrawv, KEEP_cg, op=OP.mult)
                s["KEEP"], s["PRv"] = KEEP, PRv

            def emit_rest_b(s):
                t = s["t"]
                G = s["G"]
                KEEP = s["KEEP"]
                PRv = s["PRv"]
                keep_u, keep_n, keep_d = (
                    KEEP[:, 0:C],
                    KEEP[:, C : 2 * C],
                    KEEP[:, 2 * C : 3 * C],
                )
                CN = pool.tile([P, C], U8, name="CN")
                dve.tensor_tensor(
                    CN[:], PRv[:, :, 1], PRv[:, :, 2], op=OP.is_ge
                )
                MND = pool.tile([P, C], F32, name="MND")
                dve.tensor_tensor(
                    MND[:], PRv[:, :, 1], PRv[:, :, 2], op=OP.max
                )
                CU = pool.tile([P, C], U8, name="CU")
                dve.tensor_tensor(CU[:], PRv[:, :, 0], MND[:], op=OP.is_ge)
                cnb = CN[:].broadcast_to((P, C, 3))
                cub = CU[:].broadcast_to((P, C, 3))

                OTraw = pool.tile([P, 3 * C], F32, name="OTraw")
                OTrawv = OTraw[:].rearrange("p (c j) -> p c j", j=3)
                act.copy(OTrawv, G[2])
                dve.copy_predicated(OTrawv, cnb, G[1])
                dve.copy_predicated(OTrawv, cub, G[0])

                KSEL = pool.tile([P, C], F32, name="KSEL")
                act.copy(KSEL[:], keep_d)
                dve.copy_predicated(KSEL[:], CN[:], keep_n)
                dve.copy_predicated(KSEL[:], CU[:], keep_u)

                OT = pool.tile([P, 3 * C], F32, name="OT")
                OTv = OT[:].rearrange("p (c j) -> p c j", j=3)
                kselb = KSEL[:].broadcast_to((P, C, 3))
                gp.tensor_tensor(OTv, OTrawv, kselb, op=OP.mult)
                nc.sync.dma_start(ot[t], OT[:])

            tins = {}
            fronts = {}
            tins[0] = emit_dma(0)
            fronts[0] = emit_front(0, tins.pop(0))
            emit_mid(fronts[0])
            if T > 1:
                tins[1] = emit_dma(1)
            for t in range(T):
                if t + 2 < T:
                    tins[t + 2] = emit_dma(t + 2)
                s = fronts.pop(t)
                if t + 1 in tins:
                    fronts[t + 1] = emit_front(t + 1, tins.pop(t + 1))
                emit_rest_a(s)
                if t + 1 in fronts:
                    emit_mid(fronts[t + 1])
                emit_rest_b(s)

    return nc


def build_nc_v6(R: int, C: int, bufs: int = 4):
    """v5's sign-decomposed codes + op-granular weaving: the previous
    tile's choice/output stage (rest_b) is interleaved instruction-by-
    instruction into the current tile's codes/chain/select stage, so
    the in-order DVE stream always has ready independent work between
    serial-chain ops, and the Pool PR-mult gets a full stage of slack
    before its consumers run."""
    T = R // (P * C)
    assert R == T * P * C, (R, C)
    nc = bass.Bass("TRN2", debug=False)
    x = nc.declare_dram_parameter("x", [R, 9], F32, isOutput=False)
    o = nc.declare_dram_parameter("o", [R, 3], F32, isOutput=True)
    xt = x[:].rearrange("(t p c) j -> t p (c j)", t=T, p=P, c=C)
    ot = o[:].rearrange("(t p c) j -> t p (c j)", t=T, p=P, c=C)

    dve = nc.vector
    gp = nc.gpsimd
    act = nc.scalar

    with TileContext(nc) as tc:
        with tc.tile_pool(name="pool", bufs=bufs) as pool:

            def emit_dma(t):
                tin = pool.tile([P, 9 * C], F32, name="tin")
                nc.sync.dma_start(tin[:], xt[t])
                return tin

            def emit_front(t, tin):
                s = {"t": t}
                tv = tin[:].rearrange("p (c g j) -> p c g j", c=C, g=3, j=3)
                s["V"] = [tv[:, :, :, j] for j in range(3)]
                s["G"] = [tv[:, :, g, :] for g in range(3)]
                V = s["V"]
                sigs = []
                for nm, (a, b) in (
                    ("01", (V[0], V[1])),
                    ("02", (V[0], V[2])),
                    ("21", (V[2], V[1])),
                ):
                    D = pool.tile([P, 3 * C], F32, name=f"D{nm}")
                    Dv = D[:].rearrange("p (c g) -> p c g", g=3)
                    gp.tensor_tensor(Dv, a, b, op=OP.subtract)
                    S = pool.tile([P, 3 * C], BF16, name=f"S{nm}")
                    S_gc = S[:].rearrange("p (g c) -> p g c", g=3)
                    act.sign(S_gc, D[:].rearrange("p (c g) -> p g c", g=3))
                    sigs.append(S)
                s["s01"], s["s02"], s["s21"] = sigs
                return s

            def gen_main(s):
                # codes + chain + probe selects + KEEP + PR for tile t
                V = s["V"]
                s01, s02, s21 = s["s01"], s["s02"], s["s21"]
                U = pool.tile([P, 3 * C], BF16, name="U")
                dve.tensor_tensor(U[:], s01[:], s02[:], op=OP.add)
                yield
                W = pool.tile([P, 3 * C], BF16, name="W")
                dve.tensor_tensor(W[:], s21[:], s02[:], op=OP.subtract)
                yield
                RU = pool.tile([P, 3 * C], BF16, name="RU")
                dve.tensor_scalar(
                    RU[:], U[:], -1.0, 0.0, op0=OP.add, op1=OP.max
                )
                yield
                RW = pool.tile([P, 3 * C], BF16, name="RW")
                dve.tensor_scalar(
                    RW[:], W[:], -1.0, 0.0, op0=OP.add, op1=OP.max
                )
                yield
                M = pool.tile([P, 3 * C], BF16, name="M")
                dve.tensor_tensor(M[:], RU[:], RW[:], op=OP.subtract)
                yield
                mu, mn, md = (
                    M[:, 0:C],
                    M[:, C : 2 * C],
                    M[:, 2 * C : 3 * C],
                )
                S1 = pool.tile([P, C], BF16, name="S1")
                dve.tensor_tensor(S1[:], mu, md, op=OP.add)
                yield
                S2 = pool.tile([P, C], BF16, name="S2")
                dve.tensor_tensor(S2[:], S1[:], mn, op=OP.add)
                yield
                T1 = pool.tile([P, C], BF16, name="T1")
                dve.tensor_tensor(T1[:], mn, S2[:], op=OP.mult)
                yield
                CALC = pool.tile([P, C], BF16, name="CALC")
                dve.tensor_tensor(CALC[:], mn, T1[:], op=OP.mult)
                yield
                SGN = pool.tile([P, C], BF16, name="SGN")
                act.sign(SGN[:], CALC[:])
                yield
                E0 = pool.tile([P, C], U8, name="E0")
                dve.tensor_scalar(E0[:], CALC[:], 1.0, None, op0=OP.is_equal)
                yield
                E1 = pool.tile([P, C], U8, name="E1")
                dve.tensor_scalar(E1[:], CALC[:], 0.0, None, op0=OP.is_equal)
                yield
                PRraw = pool.tile([P, 3 * C], F32, name="PRraw")
                PRrawv = PRraw[:].rearrange("p (c g) -> p c g", g=3)
                act.copy(PRrawv, V[2])
                yield
                e1b = E1[:].broadcast_to((P, C, 3))
                e0b = E0[:].broadcast_to((P, C, 3))
                dve.copy_predicated(PRrawv, e1b, V[1])
                yield
                dve.copy_predicated(PRrawv, e0b, V[0])
                yield
                KEEP = pool.tile([P, 3 * C], F32, name="KEEP")
                KEEP_cg2 = KEEP[:].rearrange("p (g c) -> p c g", g=3)
                sgn_b = SGN[:].broadcast_to((P, C, 3))
                M_cg = M[:].rearrange("p (g c) -> p c g", g=3)
                dve.tensor_tensor(KEEP_cg2, M_cg, sgn_b, op=OP.is_equal)
                yield
                PR = pool.tile([P, 3 * C], F32, name="PR")
                PRv = PR[:].rearrange("p (c g) -> p c g", g=3)
                KEEP_cg = KEEP[:].rearrange("p (g c) -> p c g", g=3)
                gp.tensor_tensor(PRv, PRrawv, KEEP_cg, op=OP.mult)
                s["KEEP"], s["PRv"] = KEEP, PRv

            def gen_tail(s):
                # choice + output for tile t (runs woven into t+1's main)
                t = s["t"]
                G = s["G"]
                KEEP = s["KEEP"]
                PRv = s["PRv"]
                keep_u, keep_n, keep_d = (
                    KEEP[:, 0:C],
                    KEEP[:, C : 2 * C],
                    KEEP[:, 2 * C : 3 * C],
                )
                CN = pool.tile([P, C], U8, name="CN")
                dve.tensor_tensor(
                    CN[:], PRv[:, :, 1], PRv[:, :, 2], op=OP.is_ge
                )
                yield
                MND = pool.tile([P, C], F32, name="MND")
                dve.tensor_tensor(
                    MND[:], PRv[:, :, 1], PRv[:, :, 2], op=OP.max
                )
                yield
                CU = pool.tile([P, C], U8, name="CU")
                dve.tensor_tensor(CU[:], PRv[:, :, 0], MND[:], op=OP.is_ge)
                yield
                cnb = CN[:].broadcast_to((P, C, 3))
                cub = CU[:].broadcast_to((P, C, 3))
                OTraw = pool.tile([P, 3 * C], F32, name="OTraw")
                OTrawv = OTraw[:].rearrange("p (c j) -> p c j", j=3)
                act.copy(OTrawv, G[2])
                yield
                dve.copy_predicated(OTrawv, cnb, G[1])
                yield
                dve.copy_predicated(OTrawv, cub, G[0])
                yield
                KSEL = pool.tile([P, C], F32, name="KSEL")
                act.copy(KSEL[:], keep_d)
                yield
                dve.copy_predicated(KSEL[:], CN[:], keep_n)
                yield
                dve.copy_predicated(KSEL[:], CU[:], keep_u)
                yield
                OT = pool.tile([P, 3 * C], F32, name="OT")
                OTv = OT[:].rearrange("p (c j) -> p c j", j=3)
                kselb = KSEL[:].broadcast_to((P, C, 3))
                gp.tensor_tensor(OTv, OTrawv, kselb, op=OP.mult)
                nc.sync.dma_start(ot[t], OT[:])

            def weave(*gens):
                live = [g for g in gens if g is not None]
                while live:
                    nxt = []
                    for g in live:
                        try:
                            next(g)
                            nxt.append(g)
                        except StopIteration:
                            pass
                    live = nxt

            def gen_front(t, tin):
                s = {"t": t}
                tv = tin[:].rearrange("p (c g j) -> p c g j", c=C, g=3, j=3)
                s["V"] = [tv[:, :, :, j] for j in range(3)]
                s["G"] = [tv[:, :, g, :] for g in range(3)]
                V = s["V"]
                sigs = {}
                for nm, (a, b) in (
                    ("01", (V[0], V[1])),
                    ("02", (V[0], V[2])),
                    ("21", (V[2], V[1])),
                ):
                    D = pool.tile([P, 3 * C], F32, name=f"D{nm}")
                    Dv = D[:].rearrange("p (c g) -> p c g", g=3)
                    gp.tensor_tensor(Dv, a, b, op=OP.subtract)
                    yield s
                    S = pool.tile([P, 3 * C], BF16, name=f"S{nm}")
                    S_gc = S[:].rearrange("p (g c) -> p g c", g=3)
                    act.sign(S_gc, D[:].rearrange("p (c g) -> p g c", g=3))
                    sigs[nm] = S
                    yield s
                s["s01"], s["s02"], s["s21"] = (
                    sigs["01"],
                    sigs["02"],
                    sigs["21"],
                )
                yield s

            def drain(g):
                out = None
                for out in g:
                    pass
                return out

            tins = {}
            fronts = {}
            tins[0] = emit_dma(0)
            fronts[0] = drain(gen_front(0, tins.pop(0)))
            if T > 1:
                tins[1] = emit_dma(1)
            prev = None
            for t in range(T):
                if t + 2 < T:
                    tins[t + 2] = emit_dma(t + 2)
                fgen = (
                    gen_front(t + 1, tins.pop(t + 1))
                    if t + 1 in tins
                    else None
                )
                s = fronts.pop(t)
                # run main(t) + tail(t-1) interleaved; drain front(t+1)
                # afterwards so its Pool subs queue behind PR(t)/OT(t-1)
                weave(gen_main(s), gen_tail(prev) if prev else None)
                if fgen is not None:
                    fronts[t + 1] = drain(fgen)
                prev = s
            weave(gen_tail(prev))

    return nc


_BUILT = {}


def _get_nc(R: int, C: int):
    key = (R, C)
    if key not in _BUILT:
        ver = os.environ.get("KERNEL_V", "4")
        build = {
            "1": build_nc,
            "2": build_nc_v2,
            "3": build_nc_v3,
            "4": build_nc_v4,
            "5": build_nc_v5,
            "6": build_nc_v6,
        }[ver]
        default_bufs = "2" if (ver == "1" or C >= 512) else "4"
        bufs = int(os.environ.get("KERNEL_BUFS", default_bufs))
        nc = build(R, C, bufs=bufs)
        split_multi_waits(nc)
        _BUILT[key] = nc
    return _BUILT[key]


KERNEL_C = int(os.environ.get("KERNEL_C", "256"))


class _Runner:
    """Jit-once shard_map runner over N_CORES cores.

    Keeps a device-resident donated output buffer chained from call to
    call: the bass kernel writes every element of ``o``, so recycling
    the previous output as the next donated buffer is safe and avoids
    shipping 100MB of zeros host->device per call.
    """

    def __init__(self, n_rows: int, C: int):
        import jax
        import jax.numpy as jnp
        from jax.experimental.shard_map import shard_map
        from jax.sharding import Mesh, NamedSharding, PartitionSpec

        from concourse import bass2jax

        bass2jax.install_neuronx_cc_hook()

        assert n_rows % N_CORES == 0
        r = n_rows // N_CORES
        nc = _get_nc(r, C)

        part_name = (
            nc.partition_id_tensor.name if nc.partition_id_tensor else None
        )
        in_names = []
        out_names = []
        out_avals = []
        for alloc in nc.m.functions[0].allocations:
            if not isinstance(alloc, mybir.MemoryLocationSet):
                continue
            name = alloc.memorylocations[0].name
            if alloc.kind == "ExternalInput":
                if name != part_name:
                    in_names.append(name)
            elif alloc.kind == "ExternalOutput":
                out_names.append(name)
                out_avals.append(
                    jax.core.ShapedArray(
                        tuple(alloc.tensor_shape), mybir.dt.np(alloc.dtype)
                    )
                )
        assert in_names == ["x"] and out_names == ["o"], (in_names, out_names)
        all_in = in_names + out_names
        if part_name is not None:
            all_in.append(part_name)
        all_in = tuple(all_in)

        def _body(xs, obuf):
            operands = [xs, obuf]
            if part_name is not None:
                operands.append(bass2jax.partition_id_tensor())
            outs = bass2jax._bass_exec_p.bind(
                *operands,
                out_avals=tuple(out_avals),
                in_names=all_in,
                out_names=tuple(out_names),
                lowering_input_output_aliases=(),
                sim_require_finite=True,
                sim_require_nnan=True,
                nc=nc,
            )
            return outs[0]

        devices = jax.devices()[:N_CORES]
        assert len(devices) == N_CORES, len(jax.devices())
        mesh = Mesh(np.asarray(devices), ("core",))
        spec = PartitionSpec("core")
        self._sharding = NamedSharding(mesh, spec)
        self._fn = jax.jit(
            shard_map(
                _body,
                mesh=mesh,
                in_specs=(spec, spec),
                out_specs=spec,
                check_rep=False,
            ),
            donate_argnums=(1,),
            keep_unused=True,
        )
        self._zeros = jax.jit(
            lambda: jnp.zeros((n_rows, 3), np.float32),
            out_shardings=self._sharding,
        )
        self._obuf = None

    def __call__(self, x: np.ndarray) -> np.ndarray:
        if self._obuf is None:
            self._obuf = self._zeros()
        out = self._fn(x, self._obuf)
        self._obuf = None  # donated; invalid from here
        res = np.asarray(out)
        self._obuf = out  # kernel writes every element: recycle as next buffer
        return res


_RUNNERS = {}


def _get_runner(n_rows: int) -> _Runner:
    key = n_rows
    if key not in _RUNNERS:
        _RUNNERS[key] = _Runner(n_rows, KERNEL_C)
    return _RUNNERS[key]


def kernel(inputs) -> np.ndarray:
    x = np.ascontiguousarray(np.asarray(inputs, dtype=np.float32))
    try:
        return _get_runner(x.shape[0])(x)
    except Exception:
        pass
    # Fallback: the stock spmd runner (slower per call: re-jits and
    # round-trips host copies, but uses only public bass_utils API).
    n = x.shape[0]
    assert n % N_CORES == 0
    r = n // N_CORES
    nc = _get_nc(r, KERNEL_C)
    shards = x.reshape(N_CORES, r, 9)
    from concourse.bass_utils import run_bass_kernel_spmd

    res = run_bass_kernel_spmd(
        nc,
        [{"x": shards[i]} for i in range(N_CORES)],
        core_ids=list(range(N_CORES)),
    )
    return np.concatenate([r_["o"] for r_ in res.results], axis=0)


# revision 38
# speedup vs baseline: 1.0447x; 1.0447x over previous
"""Trainium2 Bass kernel for nn_ConcatLayer_37589553774933 (topk_masking).

Per-row computation on [N, 9] f32 (N = 8388608): three groups of 3
(up/none/down); per group a strict-argmax code in {-1,0,1}; a scalar
decision chain (calc/sign/idx); masking; probe-argmax group selection;
output [N, 3]. Rows are split evenly across 8 NeuronCores (SPMD).

Host path: the full [N, 9] array is handed directly to a jit-once
shard_map over 8 cores (each device gets a contiguous row block, no
host-side concat); the output buffer is a device-resident donated
array recycled from the previous call's output (the kernel writes
every output element, so stale contents are harmless), and the global
[N, 3] result is fetched with a single np.asarray.
"""


import os

os.environ.setdefault("JAX_PLATFORMS", "axon,cpu")

import numpy as np

import concourse.bass as bass
import concourse.mybir as mybir
from concourse.tile import TileContext

F32 = mybir.dt.float32
BF16 = mybir.dt.bfloat16
U8 = mybir.dt.uint8
OP = mybir.AluOpType

N_TOTAL = 8388608
N_CORES = 8
R_CORE = N_TOTAL // N_CORES
P = 128
ACT_SIGN_AB = os.environ.get("ACT_SIGN_AB", "1") == "1"
INTERLEAVE_FRONT = os.environ.get("INTERLEAVE_FRONT", "1") == "1"
SPLIT_PR = os.environ.get("SPLIT_PR", "0") == "1"


def _copy_pred(eng, out, mask, data):
    return eng.add_instruction(
        mybir.InstCopyPredicated(
            name=f"I-{eng.bass.next_id()}",
            ins=[eng.lower_ap(mask, opt=False), eng.lower_ap(data, opt=False)],
            outs=[eng.lower_ap(out, opt=False)],
        )
    )


def split_multi_waits(nc, max_waits: int = 1):
    n_split = 0
    for f in nc.m.functions:
        for b in f.blocks:
            new_insts = []
            for ins in b.instructions:
                si = getattr(ins, "sync_info", None)
                if si and si.on_wait and len(si.on_wait) > max_waits:
                    waits = list(si.on_wait)
                    head, tail = waits[:-max_waits], waits[-max_waits:]
                    for k in range(0, len(head), max_waits):
                        chunk = head[k : k + max_waits]
                        nop = mybir.InstNoOp(
                            name=f"{ins.name}_waitsplit{k}",
                            ins=[],
                            outs=[],
                            sync_info=mybir.SyncInfo(
                                on_wait=list(chunk), on_update=[]
                            ),
                        )
                        nop.engine = ins.engine
                        new_insts.append(nop)
                    si.on_wait = tail
                    n_split += 1
                new_insts.append(ins)
            b.instructions = new_insts
    return n_split


def build_nc(R: int, C: int, bufs: int = 2):
    T = R // (P * C)
    assert R == T * P * C, (R, C)
    nc = bass.Bass("TRN2", debug=False)
    x = nc.declare_dram_parameter("x", [R, 9], F32, isOutput=False)
    o = nc.declare_dram_parameter("o", [R, 3], F32, isOutput=True)
    xt = x[:].rearrange("(t p c) j -> t p (c j)", t=T, p=P, c=C)
    ot = o[:].rearrange("(t p c) j -> t p (c j)", t=T, p=P, c=C)

    dve = nc.vector
    gp = nc.gpsimd
    act = nc.scalar

    with TileContext(nc) as tc:
        with tc.tile_pool(name="pool", bufs=bufs) as pool:
            for t in range(T):
                tin = pool.tile([P, 9 * C], F32, name="tin")
                nc.sync.dma_start(tin[:], xt[t])
                tv = tin[:].rearrange("p (c g j) -> p c g j", c=C, g=3, j=3)
                V = [tv[:, :, :, j] for j in range(3)]   # [P,C,3g] stride-3
                G = [tv[:, :, g, :] for g in range(3)]   # [P,C,3j] contig j

                # --- group codes ---------------------------------------
                X = pool.tile([P, 3 * C], F32, name="X")
                Xv = X[:].rearrange("p (c g) -> p c g", g=3)
                dve.tensor_tensor(Xv, V[1], V[2], op=OP.max)
                Y = pool.tile([P, 3 * C], F32, name="Y")
                Yv = Y[:].rearrange("p (c g) -> p c g", g=3)
                dve.tensor_tensor(Yv, V[0], V[1], op=OP.max)

                DA = pool.tile([P, 3 * C], F32, name="DA")
                DAv = DA[:].rearrange("p (c g) -> p c g", g=3)
                gp.tensor_tensor(DAv, V[0], Xv, op=OP.subtract)
                DB = pool.tile([P, 3 * C], F32, name="DB")
                DBv = DB[:].rearrange("p (c g) -> p c g", g=3)
                gp.tensor_tensor(DBv, V[2], Yv, op=OP.subtract)

                A = pool.tile([P, 3 * C], BF16, name="A")
                dve.tensor_scalar(A[:], DA[:], 0.0, None, op0=OP.is_gt)
                Bt = pool.tile([P, 3 * C], BF16, name="Bt")
                dve.tensor_scalar(Bt[:], DB[:], 0.0, None, op0=OP.is_gt)
                M = pool.tile([P, 3 * C], BF16, name="M")
                dve.tensor_tensor(M[:], A[:], Bt[:], op=OP.subtract)
                Mv = M[:].rearrange("p (c g) -> p c g", g=3)
                mu, mn, md = Mv[:, :, 0], Mv[:, :, 1], Mv[:, :, 2]

                # --- calc = |mn| * (mu + md + mn) ----------------------
                S1 = pool.tile([P, C], BF16, name="S1")
                dve.tensor_tensor(S1[:], mu, md, op=OP.add)
                S2 = pool.tile([P, C], BF16, name="S2")
                dve.tensor_tensor(S2[:], S1[:], mn, op=OP.add)
                T1 = pool.tile([P, C], BF16, name="T1")
                dve.tensor_tensor(T1[:], mn, S2[:], op=OP.mult)
                CALC = pool.tile([P, C], BF16, name="CALC")
                dve.tensor_tensor(CALC[:], mn, T1[:], op=OP.mult)

                SGN = pool.tile([P, C], BF16, name="SGN")
                act.sign(SGN[:], CALC[:])
                E0 = pool.tile([P, C], U8, name="E0")
                dve.tensor_scalar(E0[:], CALC[:], 1.0, None, op0=OP.is_equal)
                E1 = pool.tile([P, C], U8, name="E1")
                dve.tensor_scalar(E1[:], CALC[:], 0.0, None, op0=OP.is_equal)

                # --- keep_g = (m_g == sgn) -----------------------------
                KD = pool.tile([P, 3 * C], BF16, name="KD")
                KDv = KD[:].rearrange("p (c g) -> p c g", g=3)
                sgnb = SGN[:].broadcast_to((P, C, 3))
                gp.tensor_tensor(KDv, Mv, sgnb, op=OP.subtract)
                KEEP = pool.tile([P, 3 * C], F32, name="KEEP")
                KEEPv = KEEP[:].rearrange("p (c g) -> p c g", g=3)
                dve.tensor_scalar(KEEP[:], KD[:], 0.0, None, op0=OP.is_equal)

                # --- probe ---------------------------------------------
                PRraw = pool.tile([P, 3 * C], F32, name="PRraw")
                PRrawv = PRraw[:].rearrange("p (c g) -> p c g", g=3)
                act.copy(PRrawv, V[2])
                e1b = E1[:].broadcast_to((P, C, 3))
                e0b = E0[:].broadcast_to((P, C, 3))
                _copy_pred(dve, PRrawv, e1b, V[1])
                _copy_pred(dve, PRrawv, e0b, V[0])
                PR = pool.tile([P, 3 * C], F32, name="PR")
                PRv = PR[:].rearrange("p (c g) -> p c g", g=3)
                gp.tensor_tensor(PR[:], PRraw[:], KEEP[:], op=OP.mult)

                # --- choice --------------------------------------------
                CN = pool.tile([P, C], U8, name="CN")
                dve.tensor_tensor(CN[:], PRv[:, :, 1], PRv[:, :, 2], op=OP.is_ge)
                MND = pool.tile([P, C], F32, name="MND")
                dve.tensor_tensor(MND[:], PRv[:, :, 1], PRv[:, :, 2], op=OP.max)
                CU = pool.tile([P, C], U8, name="CU")
                dve.tensor_tensor(CU[:], PRv[:, :, 0], MND[:], op=OP.is_ge)
                cnb = CN[:].broadcast_to((P, C, 3))
                cub = CU[:].broadcast_to((P, C, 3))

                # --- output --------------------------------------------
                OTraw = pool.tile([P, 3 * C], F32, name="OTraw")
                OTrawv = OTraw[:].rearrange("p (c j) -> p c j", j=3)
                act.copy(OTrawv, G[2])
                _copy_pred(dve, OTrawv, cnb, G[1])
                _copy_pred(dve, OTrawv, cub, G[0])

                KSEL = pool.tile([P, C], F32, name="KSEL")
                act.copy(KSEL[:], KEEPv[:, :, 2])
                _copy_pred(dve, KSEL[:], CN[:], KEEPv[:, :, 1])
                _copy_pred(dve, KSEL[:], CU[:], KEEPv[:, :, 0])

                OT = pool.tile([P, 3 * C], F32, name="OT")
                OTv = OT[:].rearrange("p (c j) -> p c j", j=3)
                kselb = KSEL[:].broadcast_to((P, C, 3))
                gp.tensor_tensor(OTv, OTrawv, kselb, op=OP.mult)

                nc.sync.dma_start(ot[t], OT[:])

    return nc


def build_nc_v2(R: int, C: int, bufs: int = 3):
    """Planar-bf16 variant: A/B/M/KD/E masks stored planar (g-major) and
    packed so bf16 DVE ops hit the 2x/4x perf modes; KEEP planar f32 so
    its per-group planes are contiguous for the KSEL selects. Engine
    split mirrors v1 (Pool: subs+mask-mults, ACT: base copies + sign,
    DVE: maxes, compares, predicated selects)."""
    T = R // (P * C)
    assert R == T * P * C, (R, C)
    nc = bass.Bass("TRN2", debug=False)
    x = nc.declare_dram_parameter("x", [R, 9], F32, isOutput=False)
    o = nc.declare_dram_parameter("o", [R, 3], F32, isOutput=True)
    xt = x[:].rearrange("(t p c) j -> t p (c j)", t=T, p=P, c=C)
    ot = o[:].rearrange("(t p c) j -> t p (c j)", t=T, p=P, c=C)

    dve = nc.vector
    gp = nc.gpsimd
    act = nc.scalar

    with TileContext(nc) as tc:
        with tc.tile_pool(name="pool", bufs=bufs) as pool:
            for t in range(T):
                tin = pool.tile([P, 9 * C], F32, name="tin")
                nc.sync.dma_start(tin[:], xt[t])
                tv = tin[:].rearrange("p (c g j) -> p c g j", c=C, g=3, j=3)
                V = [tv[:, :, :, j] for j in range(3)]   # [P,C,3g] stride-3
                G = [tv[:, :, g, :] for g in range(3)]   # [P,C,3j] contig j

                # --- group codes (planar bf16 bits) --------------------
                X = pool.tile([P, 3 * C], F32, name="X")
                Xv = X[:].rearrange("p (c g) -> p c g", g=3)
                dve.tensor_tensor(Xv, V[1], V[2], op=OP.max)
                Y = pool.tile([P, 3 * C], F32, name="Y")
                Yv = Y[:].rearrange("p (c g) -> p c g", g=3)
                dve.tensor_tensor(Yv, V[0], V[1], op=OP.max)

                DA = pool.tile([P, 3 * C], F32, name="DA")
                DAv = DA[:].rearrange("p (c g) -> p c g", g=3)
                gp.tensor_tensor(DAv, V[0], Xv, op=OP.subtract)
                DB = pool.tile([P, 3 * C], F32, name="DB")
                DBv = DB[:].rearrange("p (c g) -> p c g", g=3)
                gp.tensor_tensor(DBv, V[2], Yv, op=OP.subtract)

                # planar (g-major) bf16 bit planes, packed along c
                A = pool.tile([P, 3 * C], BF16, name="A")
                A_gc = A[:].rearrange("p (g c) -> p g c", g=3)
                DA_gc = DA[:].rearrange("p (c g) -> p g c", g=3)
                B = pool.tile([P, 3 * C], BF16, name="B")
                B_gc = B[:].rearrange("p (g c) -> p g c", g=3)
                DB_gc = DB[:].rearrange("p (c g) -> p g c", g=3)
                if ACT_SIGN_AB:
                    # sign on ACT, then relu via cheap packed-bf16 ts-max on
                    # DVE: relu(sign(d)) == (d > 0) exactly.
                    SA = pool.tile([P, 3 * C], BF16, name="SA")
                    SA_gc = SA[:].rearrange("p (g c) -> p g c", g=3)
                    act.sign(SA_gc, DA_gc)
                    dve.tensor_scalar(A[:], SA[:], 0.0, None, op0=OP.max)
                    SB = pool.tile([P, 3 * C], BF16, name="SB")
                    SB_gc = SB[:].rearrange("p (g c) -> p g c", g=3)
                    act.sign(SB_gc, DB_gc)
                    dve.tensor_scalar(B[:], SB[:], 0.0, None, op0=OP.max)
                else:
                    dve.tensor_scalar(A_gc, DA_gc, 0.0, None, op0=OP.is_gt)
                    dve.tensor_scalar(B_gc, DB_gc, 0.0, None, op0=OP.is_gt)

                M = pool.tile([P, 3 * C], BF16, name="M")
                dve.tensor_tensor(M[:], A[:], B[:], op=OP.subtract)
                mu, mn, md = (M[:, 0:C], M[:, C : 2 * C], M[:, 2 * C : 3 * C])

                # --- calc = |mn| * (mu + md + mn) (packed bf16) --------
                S1 = pool.tile([P, C], BF16, name="S1")
                dve.tensor_tensor(S1[:], mu, md, op=OP.add)
                S2 = pool.tile([P, C], BF16, name="S2")
                dve.tensor_tensor(S2[:], S1[:], mn, op=OP.add)
                T1 = pool.tile([P, C], BF16, name="T1")
                dve.tensor_tensor(T1[:], mn, S2[:], op=OP.mult)
                CALC = pool.tile([P, C], BF16, name="CALC")
                dve.tensor_tensor(CALC[:], mn, T1[:], op=OP.mult)

                SGN = pool.tile([P, C], BF16, name="SGN")
                act.sign(SGN[:], CALC[:])
                E0 = pool.tile([P, C], U8, name="E0")
                dve.tensor_scalar(E0[:], CALC[:], 1.0, None, op0=OP.is_equal)
                E1 = pool.tile([P, C], U8, name="E1")
                dve.tensor_scalar(E1[:], CALC[:], 0.0, None, op0=OP.is_equal)

                # --- keep_g = (m_g == sgn), planar f32 planes ----------
                KD = pool.tile([P, 3 * C], BF16, name="KD")
                KD_cg = KD[:].rearrange("p (g c) -> p c g", g=3)
                sgn_b = SGN[:].broadcast_to((P, C, 3))
                M_cg = M[:].rearrange("p (g c) -> p c g", g=3)
                gp.tensor_tensor(KD_cg, M_cg, sgn_b, op=OP.subtract)
                KEEP = pool.tile([P, 3 * C], F32, name="KEEP")
                dve.tensor_scalar(KEEP[:], KD[:], 0.0, None, op0=OP.is_equal)
                keep_u, keep_n, keep_d = (
                    KEEP[:, 0:C],
                    KEEP[:, C : 2 * C],
                    KEEP[:, 2 * C : 3 * C],
                )
                KEEP_cg = KEEP[:].rearrange("p (g c) -> p c g", g=3)

                # --- probe ---------------------------------------------
                PRraw = pool.tile([P, 3 * C], F32, name="PRraw")
                PRrawv = PRraw[:].rearrange("p (c g) -> p c g", g=3)
                act.copy(PRrawv, V[2])
                e1b = E1[:].broadcast_to((P, C, 3))
                e0b = E0[:].broadcast_to((P, C, 3))
                dve.copy_predicated(PRrawv, e1b, V[1])
                dve.copy_predicated(PRrawv, e0b, V[0])
                PR = pool.tile([P, 3 * C], F32, name="PR")
                PRv = PR[:].rearrange("p (c g) -> p c g", g=3)
                gp.tensor_tensor(PRv, PRrawv, KEEP_cg, op=OP.mult)

                # --- choice --------------------------------------------
                CN = pool.tile([P, C], U8, name="CN")
                dve.tensor_tensor(CN[:], PRv[:, :, 1], PRv[:, :, 2], op=OP.is_ge)
                MND = pool.tile([P, C], F32, name="MND")
                dve.tensor_tensor(MND[:], PRv[:, :, 1], PRv[:, :, 2], op=OP.max)
                CU = pool.tile([P, C], U8, name="CU")
                dve.tensor_tensor(CU[:], PRv[:, :, 0], MND[:], op=OP.is_ge)
                cnb = CN[:].broadcast_to((P, C, 3))
                cub = CU[:].broadcast_to((P, C, 3))

                # --- output --------------------------------------------
                OTraw = pool.tile([P, 3 * C], F32, name="OTraw")
                OTrawv = OTraw[:].rearrange("p (c j) -> p c j", j=3)
                act.copy(OTrawv, G[2])
                dve.copy_predicated(OTrawv, cnb, G[1])
                dve.copy_predicated(OTrawv, cub, G[0])

                KSEL = pool.tile([P, C], F32, name="KSEL")
                act.copy(KSEL[:], keep_d)
                dve.copy_predicated(KSEL[:], CN[:], keep_n)
                dve.copy_predicated(KSEL[:], CU[:], keep_u)

                OT = pool.tile([P, 3 * C], F32, name="OT")
                OTv = OT[:].rearrange("p (c j) -> p c j", j=3)
                kselb = KSEL[:].broadcast_to((P, C, 3))
                gp.tensor_tensor(OTv, OTrawv, kselb, op=OP.mult)

                nc.sync.dma_start(ot[t], OT[:])

    return nc


def build_nc_v3(R: int, C: int, bufs: int = 4):
    """v2 with software-pipelined emission: stage S0 (DMA + codes) of
    tiles t+1/t+2 is emitted before stages S1/S2 of tile t, so in-order
    engines have upstream work queued while a tile waits on cross-engine
    hops (Pool PR-mult -> DVE compares etc.). Buffer ring (bufs) must
    cover the 3-stage lifetime."""
    T = R // (P * C)
    assert R == T * P * C, (R, C)
    nc = bass.Bass("TRN2", debug=False)
    x = nc.declare_dram_parameter("x", [R, 9], F32, isOutput=False)
    o = nc.declare_dram_parameter("o", [R, 3], F32, isOutput=True)
    xt = x[:].rearrange("(t p c) j -> t p (c j)", t=T, p=P, c=C)
    ot = o[:].rearrange("(t p c) j -> t p (c j)", t=T, p=P, c=C)

    dve = nc.vector
    gp = nc.gpsimd
    act = nc.scalar

    with TileContext(nc) as tc:
        with tc.tile_pool(name="pool", bufs=bufs) as pool:

            def emit_s0(t):
                s = {}
                tin = pool.tile([P, 9 * C], F32, name="tin")
                nc.sync.dma_start(tin[:], xt[t])
                tv = tin[:].rearrange("p (c g j) -> p c g j", c=C, g=3, j=3)
                s["V"] = [tv[:, :, :, j] for j in range(3)]
                s["G"] = [tv[:, :, g, :] for g in range(3)]
                V = s["V"]

                X = pool.tile([P, 3 * C], F32, name="X")
                Xv = X[:].rearrange("p (c g) -> p c g", g=3)
                dve.tensor_tensor(Xv, V[1], V[2], op=OP.max)
                Y = pool.tile([P, 3 * C], F32, name="Y")
                Yv = Y[:].rearrange("p (c g) -> p c g", g=3)
                dve.tensor_tensor(Yv, V[0], V[1], op=OP.max)

                DA = pool.tile([P, 3 * C], F32, name="DA")
                DAv = DA[:].rearrange("p (c g) -> p c g", g=3)
                gp.tensor_tensor(DAv, V[0], Xv, op=OP.subtract)
                DB = pool.tile([P, 3 * C], F32, name="DB")
                DBv = DB[:].rearrange("p (c g) -> p c g", g=3)
                gp.tensor_tensor(DBv, V[2], Yv, op=OP.subtract)

                A = pool.tile([P, 3 * C], BF16, name="A")
                DA_gc = DA[:].rearrange("p (c g) -> p g c", g=3)
                B = pool.tile([P, 3 * C], BF16, name="B")
                DB_gc = DB[:].rearrange("p (c g) -> p g c", g=3)
                SA = pool.tile([P, 3 * C], BF16, name="SA")
                SA_gc = SA[:].rearrange("p (g c) -> p g c", g=3)
                act.sign(SA_gc, DA_gc)
                dve.tensor_scalar(A[:], SA[:], 0.0, None, op0=OP.max)
                SB = pool.tile([P, 3 * C], BF16, name="SB")
                SB_gc = SB[:].rearrange("p (g c) -> p g c", g=3)
                act.sign(SB_gc, DB_gc)
                dve.tensor_scalar(B[:], SB[:], 0.0, None, op0=OP.max)

                M = pool.tile([P, 3 * C], BF16, name="M")
                dve.tensor_tensor(M[:], A[:], B[:], op=OP.subtract)
                s["M"] = M
                s["t"] = t
                return s

            def emit_s1(s):
                M = s["M"]
                V = s["V"]
                mu, mn, md = (
                    M[:, 0:C],
                    M[:, C : 2 * C],
                    M[:, 2 * C : 3 * C],
                )
                S1 = pool.tile([P, C], BF16, name="S1")
                dve.tensor_tensor(S1[:], mu, md, op=OP.add)
                S2 = pool.tile([P, C], BF16, name="S2")
                dve.tensor_tensor(S2[:], S1[:], mn, op=OP.add)
                T1 = pool.tile([P, C], BF16, name="T1")
                dve.tensor_tensor(T1[:], mn, S2[:], op=OP.mult)
                CALC = pool.tile([P, C], BF16, name="CALC")
                dve.tensor_tensor(CALC[:], mn, T1[:], op=OP.mult)

                SGN = pool.tile([P, C], BF16, name="SGN")
                act.sign(SGN[:], CALC[:])
                E0 = pool.tile([P, C], U8, name="E0")
                dve.tensor_scalar(E0[:], CALC[:], 1.0, None, op0=OP.is_equal)
                E1 = pool.tile([P, C], U8, name="E1")
                dve.tensor_scalar(E1[:], CALC[:], 0.0, None, op0=OP.is_equal)

                KD = pool.tile([P, 3 * C], BF16, name="KD")
                KD_cg = KD[:].rearrange("p (g c) -> p c g", g=3)
                sgn_b = SGN[:].broadcast_to((P, C, 3))
                M_cg = M[:].rearrange("p (g c) -> p c g", g=3)
                gp.tensor_tensor(KD_cg, M_cg, sgn_b, op=OP.subtract)
                KEEP = pool.tile([P, 3 * C], F32, name="KEEP")
                dve.tensor_scalar(KEEP[:], KD[:], 0.0, None, op0=OP.is_equal)
                s["KEEP"] = KEEP

                PRraw = pool.tile([P, 3 * C], F32, name="PRraw")
                PRrawv = PRraw[:].rearrange("p (c g) -> p c g", g=3)
                act.copy(PRrawv, V[2])
                e1b = E1[:].broadcast_to((P, C, 3))
                e0b = E0[:].broadcast_to((P, C, 3))
                dve.copy_predicated(PRrawv, e1b, V[1])
                dve.copy_predicated(PRrawv, e0b, V[0])
                PR = pool.tile([P, 3 * C], F32, name="PR")
                KEEP_cg = KEEP[:].rearrange("p (g c) -> p c g", g=3)
                PRv = PR[:].rearrange("p (c g) -> p c g", g=3)
                gp.tensor_tensor(PRv, PRrawv, KEEP_cg, op=OP.mult)
                s["PR"] = PR

            def emit_s2(s):
                PR = s["PR"]
                KEEP = s["KEEP"]
                G = s["G"]
                t = s["t"]
                PRv = PR[:].rearrange("p (c g) -> p c g", g=3)
                keep_u, keep_n, keep_d = (
                    KEEP[:, 0:C],
                    KEEP[:, C : 2 * C],
                    KEEP[:, 2 * C : 3 * C],
                )
                CN = pool.tile([P, C], U8, name="CN")
                dve.tensor_tensor(
                    CN[:], PRv[:, :, 1], PRv[:, :, 2], op=OP.is_ge
                )
                MND = pool.tile([P, C], F32, name="MND")
                dve.tensor_tensor(
                    MND[:], PRv[:, :, 1], PRv[:, :, 2], op=OP.max
                )
                CU = pool.tile([P, C], U8, name="CU")
                dve.tensor_tensor(CU[:], PRv[:, :, 0], MND[:], op=OP.is_ge)
                cnb = CN[:].broadcast_to((P, C, 3))
                cub = CU[:].broadcast_to((P, C, 3))

                OTraw = pool.tile([P, 3 * C], F32, name="OTraw")
                OTrawv = OTraw[:].rearrange("p (c j) -> p c j", j=3)
                act.copy(OTrawv, G[2])
                dve.copy_predicated(OTrawv, cnb, G[1])
                dve.copy_predicated(OTrawv, cub, G[0])

                KSEL = pool.tile([P, C], F32, name="KSEL")
                act.copy(KSEL[:], keep_d)
                dve.copy_predicated(KSEL[:], CN[:], keep_n)
                dve.copy_predicated(KSEL[:], CU[:], keep_u)

                OT = pool.tile([P, 3 * C], F32, name="OT")
                OTv = OT[:].rearrange("p (c j) -> p c j", j=3)
                kselb = KSEL[:].broadcast_to((P, C, 3))
                gp.tensor_tensor(OTv, OTrawv, kselb, op=OP.mult)
                nc.sync.dma_start(ot[t], OT[:])

            state = {}
            for t in range(T):
                state[t] = emit_s0(t)
                if t - 1 in state:
                    emit_s1(state[t - 1])
                if t - 2 in state:
                    emit_s2(state.pop(t - 2))
            if T - 1 in state:
                emit_s1(state[T - 1])
            if T - 2 in state:
                emit_s2(state.pop(T - 2))
            if T - 1 in state:
                emit_s2(state.pop(T - 1))

    return nc


def build_nc_v4(R: int, C: int, bufs: int = 4):
    """v2 with the codes front (DMA, X/Y maxes, Pool subs, ACT signs)
    software-pipelined one tile ahead and the input DMA prefetched two
    tiles ahead: when the in-order DVE stream reaches tile t's A/B bits,
    the cross-engine chain X/Y(t) -> Pool DA/DB(t) -> ACT SA/SB(t) has
    had a full iteration to drain, removing the per-tile DVE bubble."""
    T = R // (P * C)
    assert R == T * P * C, (R, C)
    nc = bass.Bass("TRN2", debug=False)
    x = nc.declare_dram_parameter("x", [R, 9], F32, isOutput=False)
    o = nc.declare_dram_parameter("o", [R, 3], F32, isOutput=True)
    xt = x[:].rearrange("(t p c) j -> t p (c j)", t=T, p=P, c=C)
    ot = o[:].rearrange("(t p c) j -> t p (c j)", t=T, p=P, c=C)

    dve = nc.vector
    gp = nc.gpsimd
    act = nc.scalar

    with TileContext(nc) as tc:
        with tc.tile_pool(name="pool", bufs=bufs) as pool:

            def emit_dma(t):
                tin = pool.tile([P, 9 * C], F32, name="tin")
                nc.sync.dma_start(tin[:], xt[t])
                return tin

            def make_views(t, tin):
                s = {"t": t}
                tv = tin[:].rearrange("p (c g j) -> p c g j", c=C, g=3, j=3)
                s["V"] = [tv[:, :, :, j] for j in range(3)]
                s["G"] = [tv[:, :, g, :] for g in range(3)]
                return s

            def emit_x(s):
                V = s["V"]
                X = pool.tile([P, 3 * C], F32, name="X")
                Xv = X[:].rearrange("p (c g) -> p c g", g=3)
                dve.tensor_tensor(Xv, V[1], V[2], op=OP.max)
                s["Xv"] = Xv

            def emit_y(s):
                V = s["V"]
                Y = pool.tile([P, 3 * C], F32, name="Y")
                Yv = Y[:].rearrange("p (c g) -> p c g", g=3)
                dve.tensor_tensor(Yv, V[0], V[1], op=OP.max)
                s["Yv"] = Yv

            def emit_subs_signs(s):
                V = s["V"]
                DA = pool.tile([P, 3 * C], F32, name="DA")
                DAv = DA[:].rearrange("p (c g) -> p c g", g=3)
                gp.tensor_tensor(DAv, V[0], s["Xv"], op=OP.subtract)
                DB = pool.tile([P, 3 * C], F32, name="DB")
                DBv = DB[:].rearrange("p (c g) -> p c g", g=3)
                gp.tensor_tensor(DBv, V[2], s["Yv"], op=OP.subtract)

                SA = pool.tile([P, 3 * C], BF16, name="SA")
                SA_gc = SA[:].rearrange("p (g c) -> p g c", g=3)
                act.sign(SA_gc, DA[:].rearrange("p (c g) -> p g c", g=3))
                SB = pool.tile([P, 3 * C], BF16, name="SB")
                SB_gc = SB[:].rearrange("p (g c) -> p g c", g=3)
                act.sign(SB_gc, DB[:].rearrange("p (c g) -> p g c", g=3))
                s["SA"], s["SB"] = SA, SB

            def emit_front(t, tin):
                s = make_views(t, tin)
                emit_x(s)
                emit_y(s)
                emit_subs_signs(s)
                return s

            def emit_rest_a(s, nxt=None):
                V = s["V"]
                A = pool.tile([P, 3 * C], BF16, name="A")
                dve.tensor_scalar(A[:], s["SA"][:], 0.0, None, op0=OP.max)
                B = pool.tile([P, 3 * C], BF16, name="B")
                dve.tensor_scalar(B[:], s["SB"][:], 0.0, None, op0=OP.max)
                M = pool.tile([P, 3 * C], BF16, name="M")
                dve.tensor_tensor(M[:], A[:], B[:], op=OP.subtract)
                mu, mn, md = (
                    M[:, 0:C],
                    M[:, C : 2 * C],
                    M[:, 2 * C : 3 * C],
                )

                S1 = pool.tile([P, C], BF16, name="S1")
                dve.tensor_tensor(S1[:], mu, md, op=OP.add)
                if nxt is not None:
                    # weave the next tile's X max between serial chain
                    # ops: independent DVE work hides the sem-ack gaps
                    # and starts the next Pool->ACT sign chain earlier.
                    emit_x(nxt)
                S2 = pool.tile([P, C], BF16, name="S2")
                dve.tensor_tensor(S2[:], S1[:], mn, op=OP.add)
                T1 = pool.tile([P, C], BF16, name="T1")
                dve.tensor_tensor(T1[:], mn, S2[:], op=OP.mult)
                if nxt is not None:
                    emit_y(nxt)
                CALC = pool.tile([P, C], BF16, name="CALC")
                dve.tensor_tensor(CALC[:], mn, T1[:], op=OP.mult)

                SGN = pool.tile([P, C], BF16, name="SGN")
                act.sign(SGN[:], CALC[:])

                E0 = pool.tile([P, C], U8, name="E0")
                dve.tensor_scalar(E0[:], CALC[:], 1.0, None, op0=OP.is_equal)
                E1 = pool.tile([P, C], U8, name="E1")
                dve.tensor_scalar(E1[:], CALC[:], 0.0, None, op0=OP.is_equal)

                PRraw = pool.tile([P, 3 * C], F32, name="PRraw")
                PRrawv = PRraw[:].rearrange("p (c g) -> p c g", g=3)
                act.copy(PRrawv, V[2])
                e1b = E1[:].broadcast_to((P, C, 3))
                e0b = E0[:].broadcast_to((P, C, 3))
                dve.copy_predicated(PRrawv, e1b, V[1])
                dve.copy_predicated(PRrawv, e0b, V[0])

                # keep = (m == sgn) in one DVE tt — no Pool round-trip
                # on the critical path to the PR mask.
                KEEP = pool.tile([P, 3 * C], F32, name="KEEP")
                KEEP_cg2 = KEEP[:].rearrange("p (g c) -> p c g", g=3)
                sgn_b = SGN[:].broadcast_to((P, C, 3))
                M_cg = M[:].rearrange("p (g c) -> p c g", g=3)
                dve.tensor_tensor(KEEP_cg2, M_cg, sgn_b, op=OP.is_equal)

                PR = pool.tile([P, 3 * C], F32, name="PR")
                PRv = PR[:].rearrange("p (c g) -> p c g", g=3)
                KEEP_cg = KEEP[:].rearrange("p (g c) -> p c g", g=3)
                if SPLIT_PR:
                    # row-halved mask mult: the first half's choice
                    # compares can start while Pool works on half two.
                    h = C // 2
                    gp.tensor_tensor(
                        PRv[:, 0:h, :],
                        PRrawv[:, 0:h, :],
                        KEEP_cg[:, 0:h, :],
                        op=OP.mult,
                    )
                    gp.tensor_tensor(
                        PRv[:, h:C, :],
                        PRrawv[:, h:C, :],
                        KEEP_cg[:, h:C, :],
                        op=OP.mult,
                    )
                else:
                    gp.tensor_tensor(PRv, PRrawv, KEEP_cg, op=OP.mult)
                s["KEEP"], s["PRv"] = KEEP, PRv

            def emit_rest_b(s):
                t = s["t"]
                G = s["G"]
                KEEP = s["KEEP"]
                PRv = s["PRv"]
                keep_u, keep_n, keep_d = (
                    KEEP[:, 0:C],
                    KEEP[:, C : 2 * C],
                    KEEP[:, 2 * C : 3 * C],
                )
                CN = pool.tile([P, C], U8, name="CN")
                MND = pool.tile([P, C], F32, name="MND")
                CU = pool.tile([P, C], U8, name="CU")
                if SPLIT_PR:
                    h = C // 2
                    for lo, hi in ((0, h), (h, C)):
                        dve.tensor_tensor(
                            CN[:, lo:hi],
                            PRv[:, lo:hi, 1],
                            PRv[:, lo:hi, 2],
                            op=OP.is_ge,
                        )
                        dve.tensor_tensor(
                            MND[:, lo:hi],
                            PRv[:, lo:hi, 1],
                            PRv[:, lo:hi, 2],
                            op=OP.max,
                        )
                        dve.tensor_tensor(
                            CU[:, lo:hi],
                            PRv[:, lo:hi, 0],
                            MND[:, lo:hi],
                            op=OP.is_ge,
                        )
                else:
                    dve.tensor_tensor(
                        CN[:], PRv[:, :, 1], PRv[:, :, 2], op=OP.is_ge
                    )
                    dve.tensor_tensor(
                        MND[:], PRv[:, :, 1], PRv[:, :, 2], op=OP.max
                    )
                    dve.tensor_tensor(
                        CU[:], PRv[:, :, 0], MND[:], op=OP.is_ge
                    )
                cnb = CN[:].broadcast_to((P, C, 3))
                cub = CU[:].broadcast_to((P, C, 3))

                OTraw = pool.tile([P, 3 * C], F32, name="OTraw")
                OTrawv = OTraw[:].rearrange("p (c j) -> p c j", j=3)
                act.copy(OTrawv, G[2])
                dve.copy_predicated(OTrawv, cnb, G[1])
                dve.copy_predicated(OTrawv, cub, G[0])

                KSEL = pool.tile([P, C], F32, name="KSEL")
                act.copy(KSEL[:], keep_d)
                dve.copy_predicated(KSEL[:], CN[:], keep_n)
                dve.copy_predicated(KSEL[:], CU[:], keep_u)

                OT = pool.tile([P, 3 * C], F32, name="OT")
                OTv = OT[:].rearrange("p (c j) -> p c j", j=3)
                kselb = KSEL[:].broadcast_to((P, C, 3))
                gp.tensor_tensor(OTv, OTrawv, kselb, op=OP.mult)
                nc.sync.dma_start(ot[t], OT[:])

            tins = {}
            fronts = {}
            if INTERLEAVE_FRONT:
                # v5 ordering: rest_a(t) fills the Pool KD latency with
                # the PRraw cpreds; front(t+1)'s X/Y maxes fill the Pool
                # PR-mult latency before rest_b(t)'s CN/CU compares.
                tins[0] = emit_dma(0)
                fronts[0] = emit_front(0, tins.pop(0))
                if T > 1:
                    tins[1] = emit_dma(1)
                for t in range(T):
                    if t + 2 < T:
                        tins[t + 2] = emit_dma(t + 2)
                    s = fronts.pop(t)
                    if t + 1 in tins:
                        nxt = make_views(t + 1, tins.pop(t + 1))
                        emit_rest_a(s, nxt=nxt)
                        emit_subs_signs(nxt)
                        fronts[t + 1] = nxt
                    else:
                        emit_rest_a(s)
                    emit_rest_b(s)
            else:
                for t in range(T):
                    if t == 0:
                        tins[0] = emit_dma(0)
                        fronts[0] = emit_front(0, tins.pop(0))
                        if T > 1:
                            tins[1] = emit_dma(1)
                    if t + 2 < T:
                        tins[t + 2] = emit_dma(t + 2)
                    if t + 1 in tins:
                        fronts[t + 1] = emit_front(t + 1, tins.pop(t + 1))
                    s = fronts.pop(t)
                    emit_rest_a(s)
                    emit_rest_b(s)

    return nc


def build_nc_v5(R: int, C: int, bufs: int = 4):
    """v4 with the X/Y maxes eliminated: per-pair signs s01=sign(v0-v1),
    s02=sign(v0-v2), s21=sign(v2-v1) are produced by Pool subs + ACT
    signs (one tile ahead), and the group code becomes pure packed-bf16
    DVE arithmetic: m = relu(s01+s02-1) - relu(s21-s02-1), each relu a
    single fused two-op tensor_scalar. Stage order per iteration t:
    dma(t+2) | front(t+1): subs+signs | rest_a(t): selects+KEEP+PR |
    mid(t+1): codes+chain (fills the Pool PR(t) latency on DVE) |
    rest_b(t): choice+output."""
    T = R // (P * C)
    assert R == T * P * C, (R, C)
    nc = bass.Bass("TRN2", debug=False)
    x = nc.declare_dram_parameter("x", [R, 9], F32, isOutput=False)
    o = nc.declare_dram_parameter("o", [R, 3], F32, isOutput=True)
    xt = x[:].rearrange("(t p c) j -> t p (c j)", t=T, p=P, c=C)
    ot = o[:].rearrange("(t p c) j -> t p (c j)", t=T, p=P, c=C)

    dve = nc.vector
    gp = nc.gpsimd
    act = nc.scalar

    with TileContext(nc) as tc:
        with tc.tile_pool(name="pool", bufs=bufs) as pool:

            def emit_dma(t):
                tin = pool.tile([P, 9 * C], F32, name="tin")
                nc.sync.dma_start(tin[:], xt[t])
                return tin

            def emit_front(t, tin):
                s = {"t": t}
                tv = tin[:].rearrange("p (c g j) -> p c g j", c=C, g=3, j=3)
                s["V"] = [tv[:, :, :, j] for j in range(3)]
                s["G"] = [tv[:, :, g, :] for g in range(3)]
                V = s["V"]

                sigs = []
                for nm, (a, b) in (
                    ("01", (V[0], V[1])),
                    ("02", (V[0], V[2])),
                    ("21", (V[2], V[1])),
                ):
                    D = pool.tile([P, 3 * C], F32, name=f"D{nm}")
                    Dv = D[:].rearrange("p (c g) -> p c g", g=3)
                    gp.tensor_tensor(Dv, a, b, op=OP.subtract)
                    S = pool.tile([P, 3 * C], BF16, name=f"S{nm}")
                    S_gc = S[:].rearrange("p (g c) -> p g c", g=3)
                    act.sign(S_gc, D[:].rearrange("p (c g) -> p g c", g=3))
                    sigs.append(S)
                s["s01"], s["s02"], s["s21"] = sigs
                return s

            def emit_mid(s):
                # group codes + calc chain, all packed bf16 on DVE
                s01, s02, s21 = s["s01"], s["s02"], s["s21"]
                U = pool.tile([P, 3 * C], BF16, name="U")
                dve.tensor_tensor(U[:], s01[:], s02[:], op=OP.add)
                W = pool.tile([P, 3 * C], BF16, name="W")
                dve.tensor_tensor(W[:], s21[:], s02[:], op=OP.subtract)
                RU = pool.tile([P, 3 * C], BF16, name="RU")
                dve.tensor_scalar(
                    RU[:], U[:], -1.0, 0.0, op0=OP.add, op1=OP.max
                )
                M = pool.tile([P, 3 * C], BF16, name="M")
                # m = relu(u-1) - relu(w-1): fold the second relu via
                # rw = max(w-1, 0) then m = ru - rw
                RW = pool.tile([P, 3 * C], BF16, name="RW")
                dve.tensor_scalar(
                    RW[:], W[:], -1.0, 0.0, op0=OP.add, op1=OP.max
                )
                dve.tensor_tensor(M[:], RU[:], RW[:], op=OP.subtract)
                mu, mn, md = (
                    M[:, 0:C],
                    M[:, C : 2 * C],
                    M[:, 2 * C : 3 * C],
                )

                S1 = pool.tile([P, C], BF16, name="S1")
                dve.tensor_tensor(S1[:], mu, md, op=OP.add)
                S2 = pool.tile([P, C], BF16, name="S2")
                dve.tensor_tensor(S2[:], S1[:], mn, op=OP.add)
                T1 = pool.tile([P, C], BF16, name="T1")
                dve.tensor_tensor(T1[:], mn, S2[:], op=OP.mult)
                CALC = pool.tile([P, C], BF16, name="CALC")
                dve.tensor_tensor(CALC[:], mn, T1[:], op=OP.mult)

                SGN = pool.tile([P, C], BF16, name="SGN")
                act.sign(SGN[:], CALC[:])
                E0 = pool.tile([P, C], U8, name="E0")
                dve.tensor_scalar(E0[:], CALC[:], 1.0, None, op0=OP.is_equal)
                E1 = pool.tile([P, C], U8, name="E1")
                dve.tensor_scalar(E1[:], CALC[:], 0.0, None, op0=OP.is_equal)
                s["M"], s["SGN"], s["E0"], s["E1"] = M, SGN, E0, E1

            def emit_rest_a(s):
                V = s["V"]
                M, SGN, E0, E1 = s["M"], s["SGN"], s["E0"], s["E1"]

                PRraw = pool.tile([P, 3 * C], F32, name="PRraw")
                PRrawv = PRraw[:].rearrange("p (c g) -> p c g", g=3)
                act.copy(PRrawv, V[2])
                e1b = E1[:].broadcast_to((P, C, 3))
                e0b = E0[:].broadcast_to((P, C, 3))
                dve.copy_predicated(PRrawv, e1b, V[1])
                dve.copy_predicated(PRrawv, e0b, V[0])

                KEEP = pool.tile([P, 3 * C], F32, name="KEEP")
                KEEP_cg2 = KEEP[:].rearrange("p (g c) -> p c g", g=3)
                sgn_b = SGN[:].broadcast_to((P, C, 3))
                M_cg = M[:].rearrange("p (g c) -> p c g", g=3)
                dve.tensor_tensor(KEEP_cg2, M_cg, sgn_b, op=OP.is_equal)

                PR = pool.tile([P, 3 * C], F32, name="PR")
                PRv = PR[:].rearrange("p (c g) -> p c g", g=3)
                KEEP_cg = KEEP[:].rearrange("p (g c) -> p c g", g=3)
                gp.tensor_tensor(PRv, PRrawv, KEEP_cg, op=OP.mult)
                s["KEEP"], s["PRv"] = KEEP, PRv

            def emit_rest_b(s):
                t = s["t"]
                G = s["G"]
                KEEP = s["KEEP"]
                PRv = s["PRv"]
                keep_u, keep_n, keep_d = (
                    KEEP[:, 0:C],
                    KEEP[:, C : 2 * C],
                    KEEP[:, 2 * C : 3 * C],
                )
                CN = pool.tile([P, C], U8, name="CN")
                dve.tensor_tensor(
                    CN[:], PRv[:, :, 1], PRv[:, :, 2], op=OP.is_ge
                )
                MND = pool.tile([P, C], F32, name="MND")
                dve.tensor_tensor(
                    MND[:], PRv[:, :, 1], PRv[:, :, 2], op=OP.max
                )
                CU = pool.tile([P, C], U8, name="CU")
                dve.tensor_tensor(CU[:], PRv[:, :, 0], MND[:], op=OP.is_ge)
                cnb = CN[:].broadcast_to((P, C, 3))
                cub = CU[:].broadcast_to((P, C, 3))

                OTraw = pool.tile([P, 3 * C], F32, name="OTraw")
                OTrawv = OTraw[:].rearrange("p (c j) -> p c j", j=3)
                act.copy(OTrawv, G[2])
                dve.copy_predicated(OTrawv, cnb, G[1])
                dve.copy_predicated(OTrawv, cub, G[0])

                KSEL = pool.tile([P, C], F32, name="KSEL")
                act.copy(KSEL[:], keep_d)
                dve.copy_predicated(KSEL[:], CN[:], keep_n)
                dve.copy_predicated(KSEL[:], CU[:], keep_u)

                OT = pool.tile([P, 3 * C], F32, name="OT")
                OTv = OT[:].rearrange("p (c j) -> p c j", j=3)
                kselb = KSEL[:].broadcast_to((P, C, 3))
                gp.tensor_tensor(OTv, OTrawv, kselb, op=OP.mult)
                nc.sync.dma_start(ot[t], OT[:])

            tins = {}
            fronts = {}
            tins[0] = emit_dma(0)
            fronts[0] = emit_front(0, tins.pop(0))
            emit_mid(fronts[0])
            if T > 1:
                tins[1] = emit_dma(1)
            for t in range(T):
                if t + 2 < T:
                    tins[t + 2] = emit_dma(t + 2)
                s = fronts.pop(t)
                if t + 1 in tins:
                    fronts[t + 1] = emit_front(t + 1, tins.pop(t + 1))
                emit_rest_a(s)
                if t + 1 in fronts:
                    emit_mid(fronts[t + 1])
                emit_rest_b(s)

    return nc


def build_nc_v6(R: int, C: int, bufs: int = 4):
    """v5's sign-decomposed codes + op-granular weaving: the previous
    tile's choice/output stage (rest_b) is interleaved instruction-by-
    instruction into the current tile's codes/chain/select stage, so
    the in-order DVE stream always has ready independent work between
    serial-chain ops, and the Pool PR-mult gets a full stage of slack
    before its consumers run."""
    T = R // (P * C)
    assert R == T * P * C, (R, C)
    nc = bass.Bass("TRN2", debug=False)
    x = nc.declare_dram_parameter("x", [R, 9], F32, isOutput=False)
    o = nc.declare_dram_parameter("o", [R, 3], F32, isOutput=True)
    xt = x[:].rearrange("(t p c) j -> t p (c j)", t=T, p=P, c=C)
    ot = o[:].rearrange("(t p c) j -> t p (c j)", t=T, p=P, c=C)

    dve = nc.vector
    gp = nc.gpsimd
    act = nc.scalar

    with TileContext(nc) as tc:
        with tc.tile_pool(name="pool", bufs=bufs) as pool:

            def emit_dma(t):
                tin = pool.tile([P, 9 * C], F32, name="tin")
                nc.sync.dma_start(tin[:], xt[t])
                return tin

            def emit_front(t, tin):
                s = {"t": t}
                tv = tin[:].rearrange("p (c g j) -> p c g j", c=C, g=3, j=3)
                s["V"] = [tv[:, :, :, j] for j in range(3)]
                s["G"] = [tv[:, :, g, :] for g in range(3)]
                V = s["V"]
                sigs = []
                for nm, (a, b) in (
                    ("01", (V[0], V[1])),
                    ("02", (V[0], V[2])),
                    ("21", (V[2], V[1])),
                ):
                    D = pool.tile([P, 3 * C], F32, name=f"D{nm}")
                    Dv = D[:].rearrange("p (c g) -> p c g", g=3)
                    gp.tensor_tensor(Dv, a, b, op=OP.subtract)
                    S = pool.tile([P, 3 * C], BF16, name=f"S{nm}")
                    S_gc = S[:].rearrange("p (g c) -> p g c", g=3)
                    act.sign(S_gc, D[:].rearrange("p (c g) -> p g c", g=3))
                    sigs.append(S)
                s["s01"], s["s02"], s["s21"] = sigs
                return s

            def gen_main(s):
                # codes + chain + probe selects + KEEP + PR for tile t
                V = s["V"]
                s01, s02, s21 = s["s01"], s["s02"], s["s21"]
                U = pool.tile([P, 3 * C], BF16, name="U")
                dve.tensor_tensor(U[:], s01[:], s02[:], op=OP.add)
                yield
                W = pool.tile([P, 3 * C], BF16, name="W")
                dve.tensor_tensor(W[:], s21[:], s02[:], op=OP.subtract)
                yield
                RU = pool.tile([P, 3 * C], BF16, name="RU")
                dve.tensor_scalar(
                    RU[:], U[:], -1.0, 0.0, op0=OP.add, op1=OP.max
                )
                yield
                RW = pool.tile([P, 3 * C], BF16, name="RW")
                dve.tensor_scalar(
                    RW[:], W[:], -1.0, 0.0, op0=OP.add, op1=OP.max
                )
                yield
                M = pool.tile([P, 3 * C], BF16, name="M")
                dve.tensor_tensor(M[:], RU[:], RW[:], op=OP.subtract)
                yield
                mu, mn, md = (
                    M[:, 0:C],
                    M[:, C : 2 * C],
                    M[:, 2 * C : 3 * C],
                )
                S1 = pool.tile([P, C], BF16, name="S1")
                dve.tensor_tensor(S1[:], mu, md, op=OP.add)
                yield
                S2 = pool.tile([P, C], BF16, name="S2")
                dve.tensor_tensor(S2[:], S1[:], mn, op=OP.add)
                yield
                T1 = pool.tile([P, C], BF16, name="T1")
                dve.tensor_tensor(T1[:], mn, S2[:], op=OP.mult)
                yield
                CALC = pool.tile([P, C], BF16, name="CALC")
                dve.tensor_tensor(CALC[:], mn, T1[:], op=OP.mult)
                yield
                SGN = pool.tile([P, C], BF16, name="SGN")
                act.sign(SGN[:], CALC[:])
                yield
                E0 = pool.tile([P, C], U8, name="E0")
                dve.tensor_scalar(E0[:], CALC[:], 1.0, None, op0=OP.is_equal)
                yield
                E1 = pool.tile([P, C], U8, name="E1")
                dve.tensor_scalar(E1[:], CALC[:], 0.0, None, op0=OP.is_equal)
                yield
                PRraw = pool.tile([P, 3 * C], F32, name="PRraw")
                PRrawv = PRraw[:].rearrange("p (c g) -> p c g", g=3)
                act.copy(PRrawv, V[2])
                yield
                e1b = E1[:].broadcast_to((P, C, 3))
                e0b = E0[:].broadcast_to((P, C, 3))
                dve.copy_predicated(PRrawv, e1b, V[1])
                yield
                dve.copy_predicated(PRrawv, e0b, V[0])
                yield
                KEEP = pool.tile([P, 3 * C], F32, name="KEEP")
                KEEP_cg2 = KEEP[:].rearrange("p (g c) -> p c g", g=3)
                sgn_b = SGN[:].broadcast_to((P, C, 3))
                M_cg = M[:].rearrange("p (g c) -> p c g", g=3)
                dve.tensor_tensor(KEEP_cg2, M_cg, sgn_b, op=OP.is_equal)
                yield
                PR = pool.tile([P, 3 * C], F32, name="PR")
                PRv = PR[:].rearrange("p (c g) -> p c g", g=3)
                KEEP_cg = KEEP[:].rearrange("p (g c) -> p c g", g=3)
                gp.tensor_tensor(PRv, PRrawv, KEEP_cg, op=OP.mult)
                s["KEEP"], s["PRv"] = KEEP, PRv

            def gen_tail(s):
                # choice + output for tile t (runs woven into t+1's main)
                t = s["t"]
                G = s["G"]
                KEEP = s["KEEP"]
                PRv = s["PRv"]
                keep_u, keep_n, keep_d = (
                    KEEP[:, 0:C],
                    KEEP[:, C : 2 * C],
                    KEEP[:, 2 * C : 3 * C],
                )
                CN = pool.tile([P, C], U8, name="CN")
                dve.tensor_tensor(
                    CN[:], PRv[:, :, 1], PRv[:, :, 2], op=OP.is_ge
                )
                yield
                MND = pool.tile([P, C], F32, name="MND")
                dve.tensor_tensor(
                    MND[:], PRv[:, :, 1], PRv[:, :, 2], op=OP.max
                )
                yield
                CU = pool.tile([P, C], U8, name="CU")
                dve.tensor_tensor(CU[:], PRv[:, :, 0], MND[:], op=OP.is_ge)
                yield
                cnb = CN[:].broadcast_to((P, C, 3))
                cub = CU[:].broadcast_to((P, C, 3))
                OTraw = pool.tile([P, 3 * C], F32, name="OTraw")
                OTrawv = OTraw[:].rearrange("p (c j) -> p c j", j=3)
                act.copy(OTrawv, G[2])
                yield
                dve.copy_predicated(OTrawv, cnb, G[1])
                yield
                dve.copy_predicated(OTrawv, cub, G[0])
                yield
                KSEL = pool.tile([P, C], F32, name="KSEL")
                act.copy(KSEL[:], keep_d)
                yield
                dve.copy_predicated(KSEL[:], CN[:], keep_n)
                yield
                dve.copy_predicated(KSEL[:], CU[:], keep_u)
                yield
                OT = pool.tile([P, 3 * C], F32, name="OT")
                OTv = OT[:].rearrange("p (c j) -> p c j", j=3)
                kselb = KSEL[:].broadcast_to((P, C, 3))
                gp.tensor_tensor(OTv, OTrawv, kselb, op=OP.mult)
                nc.sync.dma_start(ot[t], OT[:])

            def weave(*gens):
                live = [g for g in gens if g is not None]
                while live:
                    nxt = []
                    for g in live:
                        try:
                            next(g)
                            nxt.append(g)
                        except StopIteration:
                            pass
                    live = nxt

            def gen_front(t, tin):
                s = {"t": t}
                tv = tin[:].rearrange("p (c g j) -> p c g j", c=C, g=3, j=3)
                s["V"] = [tv[:, :, :, j] for j in range(3)]
                s["G"] = [tv[:, :, g, :] for g in range(3)]
                V = s["V"]
                sigs = {}
                for nm, (a, b) in (
                    ("01", (V[0], V[1])),
                    ("02", (V[0], V[2])),
                    ("21", (V[2], V[1])),
                ):
                    D = pool.tile([P, 3 * C], F32, name=f"D{nm}")
                    Dv = D[:].rearrange("p (c g) -> p c g", g=3)
                    gp.tensor_tensor(Dv, a, b, op=OP.subtract)
                    yield s
                    S = pool.tile([P, 3 * C], BF16, name=f"S{nm}")
                    S_gc = S[:].rearrange("p (g c) -> p g c", g=3)
                    act.sign(S_gc, D[:].rearrange("p (c g) -> p g c", g=3))
                    sigs[nm] = S
                    yield s
                s["s01"], s["s02"], s["s21"] = (
                    sigs["01"],
                    sigs["02"],
                    sigs["21"],
                )
                yield s

            def drain(g):
                out = None
                for out in g:
                    pass
                return out

            tins = {}
            fronts = {}
            tins[0] = emit_dma(0)
            fronts[0] = drain(gen_front(0, tins.pop(0)))
            if T > 1:
                tins[1] = emit_dma(1)
            prev = None
            for t in range(T):
                if t + 2 < T:
                    tins[t + 2] = emit_dma(t + 2)
                fgen = (
                    gen_front(t + 1, tins.pop(t + 1))
                    if t + 1 in tins
                    else None
                )
                s = fronts.pop(t)
                # run main(t) + tail(t-1) interleaved; drain front(t+1)
                # afterwards so its Pool subs queue behind PR(t)/OT(t-1)
                weave(gen_main(s), gen_tail(prev) if prev else None)
                if fgen is not None:
                    fronts[t + 1] = drain(fgen)
                prev = s
            weave(gen_tail(prev))

    return nc


_BUILT = {}


def _get_nc(R: int, C: int):
    key = (R, C)
    if key not in _BUILT:
        ver = os.environ.get("KERNEL_V", "4")
        build = {
            "1": build_nc,
            "2": build_nc_v2,
            "3": build_nc_v3,
            "4": build_nc_v4,
            "5": build_nc_v5,
            "6": build_nc_v6,
        }[ver]
        default_bufs = "2" if (ver == "1" or C >= 512) else "4"
        bufs = int(os.environ.get("KERNEL_BUFS", default_bufs))
        nc = build(R, C, bufs=bufs)
        split_multi_waits(nc)
        _BUILT[key] = nc
    return _BUILT[key]


KERNEL_C = int(os.environ.get("KERNEL_C", "256"))


class _Runner:
    """Jit-once shard_map runner over N_CORES cores.

    Keeps a device-resident donated output buffer chained from call to
    call: the bass kernel writes every element of ``o``, so recycling
    the previous output as the next donated buffer is safe and avoids
    shipping 100MB of zeros host->device per call.
    """

    def __init__(self, n_rows: int, C: int):
        import jax
        import jax.numpy as jnp
        from jax.experimental.shard_map import shard_map
        from jax.sharding import Mesh, NamedSharding, PartitionSpec

        from concourse import bass2jax

        bass2jax.install_neuronx_cc_hook()

        assert n_rows % N_CORES == 0
        r = n_rows // N_CORES
        nc = _get_nc(r, C)

        part_name = (
            nc.partition_id_tensor.name if nc.partition_id_tensor else None
        )
        in_names = []
        out_names = []
        out_avals = []
        for alloc in nc.m.functions[0].allocations:
            if not isinstance(alloc, mybir.MemoryLocationSet):
                continue
            name = alloc.memorylocations[0].name
            if alloc.kind == "ExternalInput":
                if name != part_name:
                    in_names.append(name)
            elif alloc.kind == "ExternalOutput":
                out_names.append(name)
                out_avals.append(
                    jax.core.ShapedArray(
                        tuple(alloc.tensor_shape), mybir.dt.np(alloc.dtype)
                    )
                )
        assert in_names == ["x"] and out_names == ["o"], (in_names, out_names)
        all_in = in_names + out_names
        if part_name is not None:
            all_in.append(part_name)
        all_in = tuple(all_in)

        def _body(xs, obuf):
            operands = [xs, obuf]
            if part_name is not None:
                operands.append(bass2jax.partition_id_tensor())
            outs = bass2jax._bass_exec_p.bind(
                *operands,
                out_avals=tuple(out_avals),
                in_names=all_in,
                out_names=tuple(out_names),
                lowering_input_output_aliases=(),
                sim_require_finite=True,
                sim_require_nnan=True,
                nc=nc,
            )
            return outs[0]

        devices = jax.devices()[:N_CORES]
        assert len(devices) == N_CORES, len(jax.devices())
        mesh = Mesh(np.asarray(devices), ("core",))
        spec = PartitionSpec("core")
        self._sharding = NamedSharding(mesh, spec)
        self._fn = jax.jit(
            shard_map(
                _body,
                mesh=mesh,
                in_specs=(spec, spec),
                out_specs=spec,
                check_rep=False,
            ),
            donate_argnums=(1,),
            keep_unused=True,
        )
        self._zeros = jax.jit(
            lambda: jnp.zeros((n_rows, 3), np.float32),
            out_shardings=self._sharding,
        )
        self._obuf = None

    def __call__(self, x: np.ndarray) -> np.ndarray:
        if self._obuf is None:
            self._obuf = self._zeros()
        out = self._fn(x, self._obuf)
        self._obuf = None  # donated; invalid from here
        res = np.asarray(out)
        self._obuf = out  # kernel writes every element: recycle as next buffer
        return res


_RUNNERS = {}


def _get_runner(n_rows: int) -> _Runner:
    key = n_rows
    if key not in _RUNNERS:
        _RUNNERS[key] = _Runner(n_rows, KERNEL_C)
    return _RUNNERS[key]


def kernel(inputs) -> np.ndarray:
    x = np.ascontiguousarray(np.asarray(inputs, dtype=np.float32))
    try:
        return _get_runner(x.shape[0])(x)
    except Exception:
        pass
    # Fallback: the stock spmd runner (slower per call: re-jits and
    # round-trips host copies, but uses only public bass_utils API).
    n = x.shape[0]
    assert n % N_CORES == 0
    r = n // N_CORES
    nc = _get_nc(r, KERNEL_C)
    shards = x.reshape(N_CORES, r, 9)
    from concourse.bass_utils import run_bass_kernel_spmd

    res = run_bass_kernel_spmd(
        nc,
        [{"x": shards[i]} for i in range(N_CORES)],
        core_ids=list(range(N_CORES)),
    )
    return np.concatenate([r_["o"] for r_ in res.results], axis=0)
